# revision 4
# baseline (speedup 1.0000x reference)
"""Trainium2 Bass kernel for the bidirectional Mamba MixerModel — A2A dataflow v3.

v3 over v2:
- GpSimd (Pool) runs the scans (SBUF-only; GPSIMD cannot touch PSUM), DVE
  keeps the PSUM-reading broadcast muls.
- Scans are full-batch [128, 2048] per state: no inter-chunk carry chain,
  no 96 scalar carry copies per block.
- One merged fp16 AllReduce for x_dbl per block instead of two.
- Manual activation-table loads (2 per block) instead of ~75 auto-inserted.
- fp16 dt/dtx/dA/dBu/h working set.
"""
import sys
import numpy as np

sys.path.insert(0, "/opt/trn_rl_repo")

import concourse.bass as bass  # noqa: E402,F401
import concourse.bacc as bacc  # noqa: E402
import concourse.tile as tile  # noqa: E402
from concourse import mybir  # noqa: E402
from concourse import bass_utils  # noqa: E402

F32 = mybir.dt.float32
F32R = mybir.dt.float32r
F16 = mybir.dt.float16
Alu = mybir.AluOpType
Act = mybir.ActivationFunctionType

B, L, D, DI = 2, 2048, 512, 1024
NST, KCONV, RDT, NB = 16, 4, 32, 4
NCORES = 8
T = B * L
TOK = T // NCORES          # 512 tokens per core in token-shard phases
CH = 512
NCH = L // CH              # 4 chunks per batch
NG = D // 128
NDI = DI // 128
EPS = 1e-5
LN_EXP_SET = 6             # natural_log_exp_and_others in act_info.json
SILU_SET = 18              # silu_and_others

_PROGRAM_CACHE = {}
_LAST_RESULTS = None


def _build_program(has_lnb: bool, has_nfb: bool):
    nc = bacc.Bacc("TRN2", target_bir_lowering=False, debug=False,
                   enable_asserts=False, num_devices=NCORES)

    Tn = {}
    Tn["xs"] = nc.dram_tensor("xs", [D, TOK], F32, kind="ExternalInput")
    Tn["wi"] = nc.dram_tensor("wi", [NB, 128, 64 * 128], F16, kind="ExternalInput")
    Tn["negrs"] = nc.dram_tensor("negrs", [NB, 1, 16 * 128], F16, kind="ExternalInput")
    Tn["lnbias"] = nc.dram_tensor("lnbias", [NB, 1, 16 * 128], F16, kind="ExternalInput")
    Tn["convd"] = nc.dram_tensor("convd", [NB, 128, KCONV * 128], F16, kind="ExternalInput")
    Tn["convb"] = nc.dram_tensor("convb", [NB, 128, 1], F32, kind="ExternalInput")
    Tn["wx"] = nc.dram_tensor("wx", [NB, 128, 64], F16, kind="ExternalInput")
    Tn["wdt"] = nc.dram_tensor("wdt", [NB, 32, 128], F16, kind="ExternalInput")
    Tn["bdt"] = nc.dram_tensor("bdt", [NB, 1, 128], F16, kind="ExternalInput")
    Tn["acols"] = nc.dram_tensor("acols", [NB, 128, NST], F32, kind="ExternalInput")
    Tn["dpcol"] = nc.dram_tensor("dpcol", [NB, 128, 1], F32, kind="ExternalInput")
    Tn["wo"] = nc.dram_tensor("wo", [NB, 128, 32 * 128], F16, kind="ExternalInput")
    Tn["nfw"] = nc.dram_tensor("nfw", [128, NG], F32, kind="ExternalInput")
    Tn["nfb"] = nc.dram_tensor("nfb", [128, NG], F32, kind="ExternalInput")
    Tn["identin"] = nc.dram_tensor("identin", [128, 128], F16, kind="ExternalInput")
    Tn["selbc"] = nc.dram_tensor("selbc", [64, 32 * 128], F16, kind="ExternalInput")
    Tn["outs"] = nc.dram_tensor("outs", [D, TOK], F32, kind="ExternalOutput")

    xia_in, xia_out, xd_in, xd_out, y_in, y_out = [], [], [], [], [], []
    for i in range(NB):
        xia_in.append(nc.dram_tensor(f"xia_in_{i}", [DI, TOK], F16, kind="Internal"))
        xia_out.append(nc.dram_tensor(f"xia_out_{i}", [DI, TOK], F16, kind="Internal"))
        xd_in.append(nc.dram_tensor(f"xd_in_{i}", [64, T], F16, kind="Internal"))
        xd_out.append(nc.dram_tensor(f"xd_out_{i}", [64, T], F16,
                                     kind="Internal", addr_space="Shared"))
        y_in.append(nc.dram_tensor(f"y_in_{i}", [DI, TOK], F16, kind="Internal"))
        y_out.append(nc.dram_tensor(f"y_out_{i}", [DI, TOK], F16, kind="Internal"))
    Tn["xia_in"], Tn["xia_out"] = xia_in, xia_out
    Tn["xd_in"], Tn["xd_out"] = xd_in, xd_out
    Tn["y_in"], Tn["y_out"] = y_in, y_out

    with tile.TileContext(nc) as tc:
        _emit(nc, tc, Tn, has_lnb, has_nfb)

    nc.compile()
    return nc


def _emit(nc, tc, Tn, has_lnb, has_nfb):
    import contextlib
    RG = [list(range(NCORES))]

    def load_table(set_id):
        nc.scalar.add_instruction(mybir.InstLoadActFuncSet(
            name=nc.get_next_instruction_name(), ins=[], outs=[],
            act_func_set_id=set_id))

    ctx = contextlib.ExitStack()
    with ctx:
        consts = ctx.enter_context(tc.tile_pool(name="consts", bufs=1))
        wpool = ctx.enter_context(tc.tile_pool(name="wpool", bufs=1))
        small = ctx.enter_context(tc.tile_pool(name="small", bufs=2))
        xnpool = ctx.enter_context(tc.tile_pool(name="xnpool", bufs=1))
        ygpool = ctx.enter_context(tc.tile_pool(name="ygpool", bufs=1))
        stats = ctx.enter_context(tc.tile_pool(name="stats", bufs=1))
        bigs = ctx.enter_context(tc.tile_pool(name="bigs", bufs=1))
        zpool = ctx.enter_context(tc.tile_pool(name="zpool", bufs=1))
        xcur_p = ctx.enter_context(tc.tile_pool(name="xcur", bufs=1))
        spool = ctx.enter_context(tc.tile_pool(name="spool", bufs=2))
        hpool = ctx.enter_context(tc.tile_pool(name="hpool", bufs=1))
        ytpool = ctx.enter_context(tc.tile_pool(name="ytpool", bufs=1))
        opool = ctx.enter_context(tc.tile_pool(name="opool", bufs=1))
        rowpool = ctx.enter_context(tc.tile_pool(name="rowpool", bufs=2))
        bcast = ctx.enter_context(tc.tile_pool(name="bcast", bufs=2))
        evac = ctx.enter_context(tc.tile_pool(name="evac", bufs=2))
        ps_mm = ctx.enter_context(tc.tile_pool(name="ps_mm", bufs=2, space="PSUM"))
        ps_bc = ctx.enter_context(tc.tile_pool(name="ps_bc", bufs=2, space="PSUM"))
        ps_y = ctx.enter_context(tc.tile_pool(name="ps_y", bufs=1, space="PSUM"))

        def mm(out, lhsT, rhs, **kw):
            nc.tensor.matmul(out, lhsT=lhsT, rhs=rhs, **kw)

        # ------------- constants -------------
        ident = consts.tile([128, 128], F16, tag="ident")
        nc.sync.dma_start(out=ident[:], in_=Tn["identin"].ap())
        onescol = consts.tile([128, 1], F32R, tag="onescol")
        nc.vector.memset(onescol[:].bitcast(F32), 1.0)
        ones1 = consts.tile([1, 128], F32R, tag="ones1")
        nc.vector.memset(ones1[:].bitcast(F32), 1.0)
        ones16 = consts.tile([1, CH], F16, tag="ones16")
        nc.vector.memset(ones16[:], 1.0)
        ones1h = consts.tile([1, 128], F16, tag="ones1h")
        nc.vector.memset(ones1h[:], 1.0)
        eps_sb = consts.tile([128, 1], F32, tag="eps")
        nc.vector.memset(eps_sb[:], EPS)
        nfw_sb = consts.tile([128, NG], F32, tag="nfw")
        nc.sync.dma_start(out=nfw_sb[:], in_=Tn["nfw"].ap())
        nfb_sb = consts.tile([128, NG], F32, tag="nfb")
        nc.sync.dma_start(out=nfb_sb[:], in_=Tn["nfb"].ap())

        load_table(LN_EXP_SET)

        def load_x0():
            xt = []
            for g in range(NG):
                xg = xcur_p.tile([128, TOK], F32R, tag=f"xcur{g}",
                                 name=f"xcur{g}_init")
                nc.sync.dma_start(
                    out=xg[:],
                    in_=Tn["xs"].ap()[128 * g:128 * (g + 1), :].bitcast(F32R))
                xt.append(xg)
            return xt

        x_cur = load_x0()

        def ln_head(x_tiles):
            """stats -> (m_row f32r, rstd_row f32r, mrs16 f16)"""
            s1 = ps_bc.tile([1, TOK], F32, tag="bc", name="s1")
            s2 = ps_bc.tile([1, TOK], F32, tag="bc", name="s2")
            for g in range(NG):
                xsq = small.tile([128, TOK], F32R, tag="xsq")
                nc.scalar.square(out=xsq[:], in_=x_tiles[g][:].bitcast(F32))
                mm(s1[:], lhsT=onescol[:], rhs=x_tiles[g][:],
                   start=(g == 0), stop=(g == NG - 1))
                mm(s2[:], lhsT=onescol[:], rhs=xsq[:],
                   start=(g == 0), stop=(g == NG - 1))
            m_row = stats.tile([1, TOK], F32R, tag="mrow")
            nc.vector.tensor_scalar_mul(out=m_row[:], in0=s1[:],
                                        scalar1=1.0 / D)
            mu2 = small.tile([1, TOK], F32, tag="mu2")
            nc.vector.tensor_mul(out=mu2[:], in0=m_row[:].bitcast(F32),
                                 in1=m_row[:].bitcast(F32))
            var_row = stats.tile([1, TOK], F32, tag="var")
            nc.vector.scalar_tensor_tensor(
                out=var_row[:], in0=s2[:], scalar=1.0 / D, in1=mu2[:],
                op0=Alu.mult, op1=Alu.subtract)
            nc.scalar.activation(out=var_row[:], in_=var_row[:],
                                 func=Act.Ln, bias=eps_sb[:1, :])
            rstd_row = stats.tile([1, TOK], F32R, tag="rstd")
            nc.scalar.activation(out=rstd_row[:], in_=var_row[:],
                                 func=Act.Exp, scale=-0.5)
            return m_row, rstd_row

        # ---------------- per-block loop ----------------
        for i in range(NB):
            rev = (i % 2 == 1)
            wi_sb = wpool.tile([128, 64 * 128], F16, tag="wi")
            nc.sync.dma_start(out=wi_sb[:], in_=Tn["wi"].ap()[i])
            negrs_sb = wpool.tile([1, 16 * 128], F16, tag="negrs")
            nc.sync.dma_start(out=negrs_sb[:], in_=Tn["negrs"].ap()[i])
            lnb_sb = None
            if has_lnb:
                lnb_sb = wpool.tile([1, 16 * 128], F16, tag="lnb")
                nc.sync.dma_start(out=lnb_sb[:], in_=Tn["lnbias"].ap()[i])
            convd_sb = wpool.tile([128, KCONV * 128], F16, tag="convd")
            nc.sync.dma_start(out=convd_sb[:], in_=Tn["convd"].ap()[i])
            convb_sb = wpool.tile([128, 1], F32, tag="convb")
            nc.sync.dma_start(out=convb_sb[:], in_=Tn["convb"].ap()[i])
            wx_sb = wpool.tile([128, 64], F16, tag="wx")
            nc.sync.dma_start(out=wx_sb[:], in_=Tn["wx"].ap()[i])
            wdt_sb = wpool.tile([32, 128], F16, tag="wdt")
            nc.sync.dma_start(out=wdt_sb[:], in_=Tn["wdt"].ap()[i])
            bdt_sb = wpool.tile([1, 128], F16, tag="bdt")
            nc.sync.dma_start(out=bdt_sb[:], in_=Tn["bdt"].ap()[i])
            acols_sb = wpool.tile([128, NST], F32, tag="acols")
            nc.sync.dma_start(out=acols_sb[:], in_=Tn["acols"].ap()[i])
            dpcol_sb = wpool.tile([128, 1], F32, tag="dpcol")
            nc.sync.dma_start(out=dpcol_sb[:], in_=Tn["dpcol"].ap()[i])
            wo_sb = wpool.tile([128, 32 * 128], F16, tag="wo")
            nc.sync.dma_start(out=wo_sb[:], in_=Tn["wo"].ap()[i])

            # ---- T1: LN + in-proj (token-shard), xi groups then z ----
            m_row, rstd_row = ln_head(x_cur)
            mrs16 = stats.tile([1, TOK], F16, tag="mrs16")
            nc.vector.tensor_mul(out=mrs16[:], in0=m_row[:].bitcast(F32),
                                 in1=rstd_row[:].bitcast(F32))
            rbc = ps_bc.tile([128, TOK], F32, tag="bc", name="rbc")
            mm(rbc[:], lhsT=ones1[:], rhs=rstd_row[:], start=True, stop=True)
            xn = []
            for g in range(NG):
                xng = xnpool.tile([128, TOK], F16, tag=f"xn{g}",
                                  name=f"xn{g}")
                nc.vector.tensor_mul(out=xng[:],
                                     in0=x_cur[g][:].bitcast(F32),
                                     in1=rbc[:])
                xn.append(xng)

            def inproj_group(g, psname):
                xz = ps_mm.tile([128, TOK], F32, tag="mm", name=psname)
                for kc in range(NG):
                    lh = wi_sb[:, (g * 4 + kc) * 128:(g * 4 + kc + 1) * 128]
                    mm(xz[:], lhsT=lh, rhs=xn[kc][:],
                       start=(kc == 0), stop=False)
                mm(xz[:], lhsT=negrs_sb[:, g * 128:(g + 1) * 128],
                   rhs=mrs16[:], start=False, stop=(not has_lnb))
                if has_lnb:
                    mm(xz[:], lhsT=lnb_sb[:, g * 128:(g + 1) * 128],
                       rhs=ones16[:], start=False, stop=True)
                return xz

            for g in range(NDI):          # xi rows
                xz = inproj_group(g, f"xz_xi{g}")
                xi16 = evac.tile([128, TOK], F16, tag="xi16")
                nc.vector.tensor_scalar_mul(out=xi16[:], in0=xz[:],
                                            scalar1=1.0)
                nc.sync.dma_start(
                    out=Tn["xia_in"][i].ap()[128 * g:128 * (g + 1), :],
                    in_=xi16[:])
            nc.gpsimd.collective_compute(
                "AllToAll", Alu.bypass, replica_groups=RG,
                ins=[Tn["xia_in"][i].ap()], outs=[Tn["xia_out"][i].ap()])
            load_table(SILU_SET)
            zs = []
            for g in range(NDI):          # z rows -> silu, kept in SBUF
                xz = inproj_group(NDI + g, f"xz_z{g}")
                zg = zpool.tile([128, TOK], F16, tag=f"zs{g}", name=f"zs{g}")
                nc.scalar.activation(out=zg[:], in_=xz[:], func=Act.Silu)
                zs.append(zg)

            # ---- C1: conv + Wx + AR + dt + scan (channel-shard) ----
            xipad = [bigs.tile([128, L + 6], F16, tag=f"xipad{b}",
                               name=f"xipad{b}") for b in range(B)]
            xis = [bigs.tile([128, L], F16, tag=f"xis{b}", name=f"xis{b}")
                   for b in range(B)]
            for b in range(B):
                nc.vector.memset(xipad[b][:, 0:3], 0.0)
                nc.vector.memset(xipad[b][:, L + 3:L + 6], 0.0)
            for s in range(NCORES):
                b, q = s // NCH, s % NCH
                nc.sync.dma_start(
                    out=xipad[b][:, 3 + CH * q: 3 + CH * (q + 1)],
                    in_=Tn["xia_out"][i].ap()[128 * s:128 * (s + 1), :])

            xd16 = bigs.tile([64, T], F16, tag="xd16", name="xd16")
            for b in range(B):
                for c in range(NCH):
                    t0 = c * CH
                    cv = ps_mm.tile([128, CH], F32, tag="mm")
                    for kk in range(KCONV):
                        off = t0 + (kk if not rev else (6 - kk))
                        mm(cv[:], lhsT=convd_sb[:, kk * 128:(kk + 1) * 128],
                           rhs=xipad[b][:, off: off + CH],
                           start=(kk == 0), stop=(kk == KCONV - 1))
                    nc.scalar.activation(out=xis[b][:, t0:t0 + CH], in_=cv[:],
                                         func=Act.Silu, bias=convb_sb[:])
                    wxp = ps_mm.tile([64, CH], F32, tag="mm", name="wxp")
                    mm(wxp[:], lhsT=wx_sb[:], rhs=xis[b][:, t0:t0 + CH],
                       start=True, stop=True)
                    nc.scalar.copy(out=xd16[:, b * L + t0:b * L + t0 + CH],
                                   in_=wxp[:])
                    nc.sync.dma_start(
                        out=Tn["xd_in"][i].ap()[:, b * L + t0:b * L + t0 + CH],
                        in_=xd16[:, b * L + t0:b * L + t0 + CH])
            nc.gpsimd.collective_compute(
                "AllReduce", Alu.add, replica_groups=RG,
                ins=[Tn["xd_in"][i].ap()], outs=[Tn["xd_out"][i].ap()])

            dtr32 = bigs.tile([32, T], F16, tag="dtr32", name="dtr32")
            nc.sync.dma_start(out=dtr32[:], in_=Tn["xd_out"][i].ap()[0:32, :])

            load_table(LN_EXP_SET)
            dt = [bigs.tile([128, L], F16, tag=f"dt{b}", name=f"dt{b}")
                  for b in range(B)]
            dtx = [bigs.tile([128, L], F16, tag=f"dtx{b}", name=f"dtx{b}")
                   for b in range(B)]
            for b in range(B):
                for c in range(NCH):
                    t0 = c * CH
                    gt0 = b * L + t0
                    dt_ps = ps_mm.tile([128, CH], F32, tag="mm",
                                       name="dt_ps")
                    mm(dt_ps[:], lhsT=wdt_sb[:], rhs=dtr32[:, gt0:gt0 + CH],
                       start=True, stop=False)
                    mm(dt_ps[:], lhsT=bdt_sb[:], rhs=ones16[:],
                       start=False, stop=True)
                    e_sb = small.tile([128, CH], F32, tag="sp_e", name="e_sb")
                    nc.scalar.activation(out=e_sb[:], in_=dt_ps[:],
                                         func=Act.Exp, scale=-1.0)
                    nc.scalar.activation(out=e_sb[:], in_=e_sb[:],
                                         func=Act.Ln, bias=1.0)
                    dtr = small.tile([128, CH], F32, tag="dtr", name="dtr")
                    nc.scalar.copy(out=dtr[:], in_=dt_ps[:])
                    nc.vector.tensor_add(out=dt[b][:, t0:t0 + CH],
                                         in0=dtr[:], in1=e_sb[:])
                    nc.vector.tensor_mul(out=dtx[b][:, t0:t0 + CH],
                                         in0=dt[b][:, t0:t0 + CH],
                                         in1=xis[b][:, t0:t0 + CH])

            # ---- scan: full-batch per state; Pool scans, DVE muls ----
            for b in range(B):
                y_ps = [ps_y.tile([128, CH], F32, tag=f"y{c}", name=f"y{c}")
                        for c in range(NCH)]

                def revfull(tl):
                    return tl[:, L - 1::-1] if rev else tl[:, 0:L]

                def revchunk(tl, t0o):
                    if not rev:
                        return tl[:, t0o:t0o + CH]
                    if t0o == 0:
                        return tl[:, CH - 1::-1]
                    return tl[:, t0o + CH - 1:t0o - 1:-1]

                for n in range(NST):
                    pe_route = (n % 8 == 7)
                    dA = spool.tile([128, L], F16, tag="dA")
                    nc.scalar.activation(out=dA[:], in_=revfull(dt[b]),
                                         func=Act.Exp,
                                         scale=acols_sb[:, n:n + 1])
                    brow = rowpool.tile([1, L], F16, tag="brow")
                    nc.sync.dma_start(
                        out=brow[:],
                        in_=Tn["xd_out"][i].ap()[32 + n:33 + n,
                                                 b * L:(b + 1) * L])
                    dBu = spool.tile([128, L], F16, tag="dBu")
                    if pe_route:
                        for c in range(NCH):
                            co = (NCH - 1 - c) if rev else c
                            t0o = co * CH
                            bbc = ps_bc.tile([128, CH], F32, tag="bc")
                            mm(bbc[:], lhsT=ones1h[:],
                               rhs=brow[0:1, t0o:t0o + CH],
                               start=True, stop=True)
                            bbc_r = bbc[:, CH - 1::-1] if rev else bbc[:]
                            nc.vector.tensor_mul(
                                out=dBu[:, c * CH:(c + 1) * CH],
                                in0=revchunk(dtx[b], t0o), in1=bbc_r)
                    else:
                        bbc_sb = bcast.tile([128, L], F16, tag="bbc")
                        nc.gpsimd.partition_broadcast(bbc_sb[:], brow[0:1, :])
                        nc.vector.tensor_mul(out=dBu[:], in0=revfull(dtx[b]),
                                             in1=revfull(bbc_sb))
                    h = hpool.tile([128, L], F16, tag="h")
                    nc.vector.tensor_tensor_scan(h[:], dA[:], dBu[:], 0.0,
                                                 op0=Alu.mult, op1=Alu.add)
                    crow = rowpool.tile([1, L], F16, tag="crow")
                    nc.sync.dma_start(
                        out=crow[:],
                        in_=Tn["xd_out"][i].ap()[48 + n:49 + n,
                                                 b * L:(b + 1) * L])
                    yterm = ytpool.tile([128, L], F16, tag="yterm")
                    if pe_route:
                        for c in range(NCH):
                            co = (NCH - 1 - c) if rev else c
                            t0o = co * CH
                            cbc = ps_bc.tile([128, CH], F32, tag="bc")
                            mm(cbc[:], lhsT=ones1h[:],
                               rhs=crow[0:1, t0o:t0o + CH],
                               start=True, stop=True)
                            cbc_r = cbc[:, CH - 1::-1] if rev else cbc[:]
                            nc.vector.tensor_mul(
                                out=yterm[:, c * CH:(c + 1) * CH],
                                in0=h[:, c * CH:(c + 1) * CH], in1=cbc_r)
                    else:
                        cbc_sb = bcast.tile([128, L], F16, tag="cbc")
                        nc.gpsimd.partition_broadcast(cbc_sb[:], crow[0:1, :])
                        nc.vector.tensor_mul(out=yterm[:], in0=h[:],
                                             in1=revfull(cbc_sb))
                    for c in range(NCH):
                        mm(y_ps[c][:], lhsT=ident[:],
                           rhs=yterm[:, c * CH:(c + 1) * CH],
                           start=(n == 0), stop=(n == NST - 1))
                for c in range(NCH):
                    co = (NCH - 1 - c) if rev else c
                    t0o = co * CH
                    gc = NCH * b + co
                    y16 = evac.tile([128, CH], F16, tag="y16")
                    yout = y16[:, CH - 1::-1] if rev else y16[:]
                    nc.scalar.copy(out=yout, in_=y_ps[c][:])
                    yfin = evac.tile([128, CH], F16, tag="yfin")
                    nc.vector.scalar_tensor_tensor(
                        out=yfin[:], in0=xis[b][:, t0o:t0o + CH],
                        scalar=dpcol_sb[:], in1=y16[:],
                        op0=Alu.mult, op1=Alu.add)
                    nc.sync.dma_start(
                        out=Tn["y_in"][i].ap()[128 * gc:128 * (gc + 1), :],
                        in_=yfin[:])
            nc.gpsimd.collective_compute(
                "AllToAll", Alu.bypass, replica_groups=RG,
                ins=[Tn["y_in"][i].ap()], outs=[Tn["y_out"][i].ap()])

            # ---- T2: gate + out-proj (token-shard) ----
            yg = []
            for s in range(NDI):
                ydn = evac.tile([128, TOK], F16, tag="ydn")
                nc.sync.dma_start(
                    out=ydn[:],
                    in_=Tn["y_out"][i].ap()[128 * s:128 * (s + 1), :])
                ygs = ygpool.tile([128, TOK], F16, tag=f"yg{s}",
                                  name=f"yg{s}")
                nc.vector.tensor_mul(out=ygs[:], in0=ydn[:], in1=zs[s][:])
                yg.append(ygs)
            x_next = []
            for g in range(NG):
                op_ps = ps_mm.tile([128, TOK], F32, tag="mm")
                for kc in range(NDI):
                    lh = wo_sb[:, (g * 8 + kc) * 128:(g * 8 + kc + 1) * 128]
                    mm(op_ps[:], lhsT=lh, rhs=yg[kc][:],
                       start=(kc == 0), stop=(kc == NDI - 1))
                xg = xcur_p.tile([128, TOK], F32R, tag=f"xcur{g}",
                                 name=f"xcur{g}_{i}")
                nc.scalar.copy(out=xg[:], in_=op_ps[:])
                x_next.append(xg)
            x_cur = x_next

        # ---------------- final layernorm (token-shard) ----------------
        m_row, rstd_row = ln_head(x_cur)
        mbc = ps_bc.tile([128, TOK], F32, tag="bc", name="mbc")
        mm(mbc[:], lhsT=ones1[:], rhs=m_row[:], start=True, stop=True)
        rbc_ps = ps_bc.tile([128, TOK], F32, tag="bc", name="rbcf")
        mm(rbc_ps[:], lhsT=ones1[:], rhs=rstd_row[:], start=True, stop=True)
        rbc = small.tile([128, TOK], F32, tag="rbc")
        nc.scalar.copy(out=rbc[:], in_=rbc_ps[:])
        for g in range(NG):
            t1_sb = small.tile([128, TOK], F32, tag="xsq", name="t1_sb")
            nc.vector.tensor_sub(out=t1_sb[:],
                                 in0=x_cur[g][:].bitcast(F32), in1=mbc[:])
            o_sb = opool.tile([128, TOK], F32, tag="o_sb")
            nc.vector.scalar_tensor_tensor(
                out=o_sb[:], in0=t1_sb[:], scalar=nfw_sb[:, g:g + 1],
                in1=rbc[:], op0=Alu.mult, op1=Alu.mult)
            if has_nfb:
                nc.vector.tensor_scalar_add(
                    out=o_sb[:], in0=o_sb[:], scalar1=nfb_sb[:, g:g + 1])
            nc.sync.dma_start(
                out=Tn["outs"].ap()[128 * g:128 * (g + 1), :], in_=o_sb[:])


def _host_prep(inputs):
    x = np.asarray(inputs["x"], np.float32)
    ln_w = np.asarray(inputs["ln_w"], np.float32)
    ln_b = np.asarray(inputs["ln_b"], np.float32)
    W_in = np.asarray(inputs["W_in"], np.float32)
    conv_w = np.asarray(inputs["conv_w"], np.float32)
    conv_b = np.asarray(inputs["conv_b"], np.float32)
    W_x = np.asarray(inputs["W_x"], np.float32)
    W_dt = np.asarray(inputs["W_dt"], np.float32)
    b_dt = np.asarray(inputs["b_dt"], np.float32)
    A_log = np.asarray(inputs["A_log"], np.float32)
    D_p = np.asarray(inputs["D_p"], np.float32)
    W_out = np.asarray(inputs["W_out"], np.float32)
    normf_w = np.asarray(inputs["normf_w"], np.float32)
    normf_b = np.asarray(inputs["normf_b"], np.float32)

    xT = np.ascontiguousarray(x.transpose(2, 0, 1).reshape(D, T))
    A = -np.exp(A_log)

    wi_arr = np.zeros((NB, 128, 64 * 128), np.float16)
    negrs_arr = np.zeros((NB, 1, 16 * 128), np.float16)
    lnb_arr = np.zeros((NB, 1, 16 * 128), np.float16)
    wo_arr = np.zeros((NB, 128, 32 * 128), np.float16)
    for i in range(NB):
        Wf = W_in[i] * ln_w[i][None, :]
        for g in range(16):
            rows = slice(128 * g, 128 * (g + 1))
            for kc in range(4):
                cols = slice(128 * kc, 128 * (kc + 1))
                wi_arr[i, :, (g * 4 + kc) * 128:(g * 4 + kc + 1) * 128] = \
                    Wf[rows, cols].T
            negrs_arr[i, 0, g * 128:(g + 1) * 128] = -Wf[rows, :].sum(1)
            lnb_arr[i, 0, g * 128:(g + 1) * 128] = W_in[i][rows, :] @ ln_b[i]
        for g in range(4):
            rows = slice(128 * g, 128 * (g + 1))
            for kc in range(8):
                cols = slice(128 * kc, 128 * (kc + 1))
                wo_arr[i, :, (g * 8 + kc) * 128:(g * 8 + kc + 1) * 128] = \
                    W_out[i][rows, cols].T

    selbc = np.zeros((64, 32 * 128), np.float16)
    for q in range(32):
        selbc[32 + q, q * 128:(q + 1) * 128] = 1.0

    nfw = np.ascontiguousarray(normf_w.reshape(NG, 128).T)
    nfb = np.ascontiguousarray(normf_b.reshape(NG, 128).T)
    identin = np.eye(128, dtype=np.float16)

    in_maps = []
    for k in range(NCORES):
        sl = slice(128 * k, 128 * (k + 1))
        convd_arr = np.zeros((NB, 128, KCONV * 128), np.float16)
        convb_arr = np.zeros((NB, 128, 1), np.float32)
        wx_arr = np.zeros((NB, 128, 64), np.float16)
        wdt_arr = np.zeros((NB, 32, 128), np.float16)
        bdt_arr = np.zeros((NB, 1, 128), np.float16)
        acols_arr = np.zeros((NB, 128, NST), np.float32)
        dp_arr = np.zeros((NB, 128, 1), np.float32)
        for i in range(NB):
            for kk in range(KCONV):
                np.fill_diagonal(convd_arr[i, :, kk * 128:(kk + 1) * 128],
                                 conv_w[i, sl, kk])
            convb_arr[i, :, 0] = conv_b[i, sl]
            wx_arr[i] = W_x[i][:, sl].T
            wdt_arr[i] = W_dt[i][sl, :].T
            bdt_arr[i, 0, :] = b_dt[i, sl]
            acols_arr[i] = A[i, sl, :]
            dp_arr[i, :, 0] = D_p[i, sl]
        in_maps.append({
            "xs": np.ascontiguousarray(xT[:, TOK * k:TOK * (k + 1)]),
            "wi": wi_arr, "negrs": negrs_arr, "lnbias": lnb_arr,
            "convd": convd_arr, "convb": convb_arr,
            "wx": wx_arr, "wdt": wdt_arr, "bdt": bdt_arr,
            "acols": acols_arr, "dpcol": dp_arr, "wo": wo_arr,
            "nfw": nfw, "nfb": nfb, "identin": identin, "selbc": selbc,
        })
    has_lnb = bool(np.any(ln_b != 0.0))
    has_nfb = bool(np.any(normf_b != 0.0))
    return in_maps, has_lnb, has_nfb


def _get_program(has_lnb, has_nfb):
    key = (has_lnb, has_nfb)
    if key not in _PROGRAM_CACHE:
        _PROGRAM_CACHE[key] = _build_program(has_lnb, has_nfb)
    return _PROGRAM_CACHE[key]


def kernel(**inputs) -> np.ndarray:
    global _LAST_RESULTS
    in_maps, has_lnb, has_nfb = _host_prep(inputs)
    nc = _get_program(has_lnb, has_nfb)
    res = bass_utils.run_bass_kernel_spmd(nc, in_maps,
                                          core_ids=list(range(NCORES)))
    _LAST_RESULTS = res
    out_T = np.concatenate([res.results[k]["outs"] for k in range(NCORES)],
                           axis=1)
    out = out_T.reshape(D, B, L).transpose(1, 2, 0)
    return np.ascontiguousarray(out.astype(np.float32))


# revision 5
# speedup vs baseline: 1.0151x; 1.0151x over previous
"""Trainium2 Bass kernel for the bidirectional Mamba MixerModel (AllToAll dataflow).

Sharding alternates per block between token-sharding (each of 8 cores owns 512
global tokens, full model width: LayerNorm, in-proj, gating, out-proj) and
channel-sharding (each core owns 128 of 1024 d_inner channels, all 4096
tokens: conv, selective scan).  The layouts are bridged by two fp16 1MB
AllToAll transposes plus one fp16 512KB AllReduce (x_dbl) per block —
replacing Megatron-style 4MB AllReduces, which dominated the baseline.

The inter-block sequence flip is pure relabeling: activations stay in original
token coordinates; odd blocks run conv taps shifted the other way and the scan
over reversed access patterns (no data movement, no extra collectives).

Scan engine placement: scans are full-batch [128, 2048] DVE ops (one per
state, no inter-chunk carry chain; fp32 recurrence state, fp16 operands).
B/C row broadcasts run as GpSimd partition_broadcast into fp16 SBUF (14 of 16
states), so the dBu/yterm multiplies hit the DVE 2-byte 2x mode; the
remaining states use a PE rank-1 broadcast via PSUM to balance Pool vs DVE.
Activation-table loads are emitted manually (2 per block: exp/ln <-> silu).

Weights ride in fp16 (in/out projections fully replicated per core for the
token-shard phases); verified end-to-end rel err ~3.1e-3 vs the fp32
reference (tolerance 2e-2).
"""
import sys
import numpy as np

sys.path.insert(0, "/opt/trn_rl_repo")

import concourse.bass as bass  # noqa: E402,F401
import concourse.bacc as bacc  # noqa: E402
import concourse.tile as tile  # noqa: E402
from concourse import mybir  # noqa: E402
from concourse import bass_utils  # noqa: E402

F32 = mybir.dt.float32
F32R = mybir.dt.float32r
F16 = mybir.dt.float16
Alu = mybir.AluOpType
Act = mybir.ActivationFunctionType

B, L, D, DI = 2, 2048, 512, 1024
NST, KCONV, RDT, NB = 16, 4, 32, 4
NCORES = 8
T = B * L
TOK = T // NCORES          # 512 tokens per core in token-shard phases
CH = 512
NCH = L // CH              # 4 chunks per batch
NG = D // 128
NDI = DI // 128
EPS = 1e-5
LN_EXP_SET = 6             # natural_log_exp_and_others in act_info.json
SILU_SET = 18              # silu_and_others

_PROGRAM_CACHE = {}
_LAST_RESULTS = None


def _build_program(has_lnb: bool, has_nfb: bool):
    nc = bacc.Bacc("TRN2", target_bir_lowering=False, debug=False,
                   enable_asserts=False, num_devices=NCORES)

    Tn = {}
    Tn["xs"] = nc.dram_tensor("xs", [D, TOK], F32, kind="ExternalInput")
    Tn["wi"] = nc.dram_tensor("wi", [NB, 128, 64 * 128], F16, kind="ExternalInput")
    Tn["negrs"] = nc.dram_tensor("negrs", [NB, 1, 16 * 128], F16, kind="ExternalInput")
    Tn["lnbias"] = nc.dram_tensor("lnbias", [NB, 1, 16 * 128], F16, kind="ExternalInput")
    Tn["convd"] = nc.dram_tensor("convd", [NB, 128, KCONV * 128], F16, kind="ExternalInput")
    Tn["convb"] = nc.dram_tensor("convb", [NB, 128, 1], F32, kind="ExternalInput")
    Tn["wx"] = nc.dram_tensor("wx", [NB, 128, 64], F16, kind="ExternalInput")
    Tn["wdt"] = nc.dram_tensor("wdt", [NB, 32, 128], F16, kind="ExternalInput")
    Tn["bdt"] = nc.dram_tensor("bdt", [NB, 1, 128], F16, kind="ExternalInput")
    Tn["acols"] = nc.dram_tensor("acols", [NB, 128, NST], F32, kind="ExternalInput")
    Tn["dpcol"] = nc.dram_tensor("dpcol", [NB, 128, 1], F32, kind="ExternalInput")
    Tn["wo"] = nc.dram_tensor("wo", [NB, 128, 32 * 128], F16, kind="ExternalInput")
    Tn["nfw"] = nc.dram_tensor("nfw", [128, NG], F32, kind="ExternalInput")
    Tn["nfb"] = nc.dram_tensor("nfb", [128, NG], F32, kind="ExternalInput")
    Tn["identin"] = nc.dram_tensor("identin", [128, 128], F16, kind="ExternalInput")
    Tn["selbc"] = nc.dram_tensor("selbc", [64, 32 * 128], F16, kind="ExternalInput")
    Tn["outs"] = nc.dram_tensor("outs", [D, TOK], F32, kind="ExternalOutput")

    xia_in, xia_out, xd_in, xd_out, y_in, y_out = [], [], [], [], [], []
    for i in range(NB):
        xia_in.append(nc.dram_tensor(f"xia_in_{i}", [DI, TOK], F16, kind="Internal"))
        xia_out.append(nc.dram_tensor(f"xia_out_{i}", [DI, TOK], F16, kind="Internal"))
        xd_in.append(nc.dram_tensor(f"xd_in_{i}", [64, T], F16, kind="Internal"))
        xd_out.append(nc.dram_tensor(f"xd_out_{i}", [64, T], F16,
                                     kind="Internal", addr_space="Shared"))
        y_in.append(nc.dram_tensor(f"y_in_{i}", [DI, TOK], F16, kind="Internal"))
        y_out.append(nc.dram_tensor(f"y_out_{i}", [DI, TOK], F16, kind="Internal"))
    Tn["xia_in"], Tn["xia_out"] = xia_in, xia_out
    Tn["xd_in"], Tn["xd_out"] = xd_in, xd_out
    Tn["y_in"], Tn["y_out"] = y_in, y_out

    with tile.TileContext(nc) as tc:
        _emit(nc, tc, Tn, has_lnb, has_nfb)

    nc.compile()
    return nc


def _emit(nc, tc, Tn, has_lnb, has_nfb):
    import contextlib
    RG = [list(range(NCORES))]

    def load_table(set_id):
        nc.scalar.add_instruction(mybir.InstLoadActFuncSet(
            name=nc.get_next_instruction_name(), ins=[], outs=[],
            act_func_set_id=set_id))

    ctx = contextlib.ExitStack()
    with ctx:
        consts = ctx.enter_context(tc.tile_pool(name="consts", bufs=1))
        wpool = ctx.enter_context(tc.tile_pool(name="wpool", bufs=1))
        small = ctx.enter_context(tc.tile_pool(name="small", bufs=2))
        xnpool = ctx.enter_context(tc.tile_pool(name="xnpool", bufs=1))
        ygpool = ctx.enter_context(tc.tile_pool(name="ygpool", bufs=1))
        stats = ctx.enter_context(tc.tile_pool(name="stats", bufs=1))
        bigs = ctx.enter_context(tc.tile_pool(name="bigs", bufs=1))
        zpool = ctx.enter_context(tc.tile_pool(name="zpool", bufs=1))
        xcur_p = ctx.enter_context(tc.tile_pool(name="xcur", bufs=1))
        spool = ctx.enter_context(tc.tile_pool(name="spool", bufs=2))
        hpool = ctx.enter_context(tc.tile_pool(name="hpool", bufs=1))
        ytpool = ctx.enter_context(tc.tile_pool(name="ytpool", bufs=1))
        opool = ctx.enter_context(tc.tile_pool(name="opool", bufs=1))
        rowpool = ctx.enter_context(tc.tile_pool(name="rowpool", bufs=2))
        bcast = ctx.enter_context(tc.tile_pool(name="bcast", bufs=3))
        evac = ctx.enter_context(tc.tile_pool(name="evac", bufs=2))
        ps_mm = ctx.enter_context(tc.tile_pool(name="ps_mm", bufs=2, space="PSUM"))
        ps_bc = ctx.enter_context(tc.tile_pool(name="ps_bc", bufs=2, space="PSUM"))
        ps_y = ctx.enter_context(tc.tile_pool(name="ps_y", bufs=1, space="PSUM"))

        def mm(out, lhsT, rhs, **kw):
            nc.tensor.matmul(out, lhsT=lhsT, rhs=rhs, **kw)

        # ------------- constants -------------
        ident = consts.tile([128, 128], F16, tag="ident")
        nc.sync.dma_start(out=ident[:], in_=Tn["identin"].ap())
        onescol = consts.tile([128, 1], F32R, tag="onescol")
        nc.vector.memset(onescol[:].bitcast(F32), 1.0)
        ones1 = consts.tile([1, 128], F32R, tag="ones1")
        nc.vector.memset(ones1[:].bitcast(F32), 1.0)
        ones16 = consts.tile([1, CH], F16, tag="ones16")
        nc.vector.memset(ones16[:], 1.0)
        ones1h = consts.tile([1, 128], F16, tag="ones1h")
        nc.vector.memset(ones1h[:], 1.0)
        eps_sb = consts.tile([128, 1], F32, tag="eps")
        nc.vector.memset(eps_sb[:], EPS)
        nfw_sb = consts.tile([128, NG], F32, tag="nfw")
        nc.sync.dma_start(out=nfw_sb[:], in_=Tn["nfw"].ap())
        nfb_sb = consts.tile([128, NG], F32, tag="nfb")
        nc.sync.dma_start(out=nfb_sb[:], in_=Tn["nfb"].ap())

        load_table(LN_EXP_SET)

        def load_x0():
            xt = []
            for g in range(NG):
                xg = xcur_p.tile([128, TOK], F32R, tag=f"xcur{g}",
                                 name=f"xcur{g}_init")
                nc.sync.dma_start(
                    out=xg[:],
                    in_=Tn["xs"].ap()[128 * g:128 * (g + 1), :].bitcast(F32R))
                xt.append(xg)
            return xt

        x_cur = load_x0()

        def ln_head(x_tiles):
            """stats -> (m_row f32r, rstd_row f32r, mrs16 f16)"""
            s1 = ps_bc.tile([1, TOK], F32, tag="bc", name="s1")
            s2 = ps_bc.tile([1, TOK], F32, tag="bc", name="s2")
            for g in range(NG):
                xsq = small.tile([128, TOK], F32R, tag="xsq")
                nc.scalar.square(out=xsq[:], in_=x_tiles[g][:].bitcast(F32))
                mm(s1[:], lhsT=onescol[:], rhs=x_tiles[g][:],
                   start=(g == 0), stop=(g == NG - 1))
                mm(s2[:], lhsT=onescol[:], rhs=xsq[:],
                   start=(g == 0), stop=(g == NG - 1))
            m_row = stats.tile([1, TOK], F32R, tag="mrow")
            nc.vector.tensor_scalar_mul(out=m_row[:], in0=s1[:],
                                        scalar1=1.0 / D)
            mu2 = small.tile([1, TOK], F32, tag="mu2")
            nc.vector.tensor_mul(out=mu2[:], in0=m_row[:].bitcast(F32),
                                 in1=m_row[:].bitcast(F32))
            var_row = stats.tile([1, TOK], F32, tag="var")
            nc.vector.scalar_tensor_tensor(
                out=var_row[:], in0=s2[:], scalar=1.0 / D, in1=mu2[:],
                op0=Alu.mult, op1=Alu.subtract)
            nc.scalar.activation(out=var_row[:], in_=var_row[:],
                                 func=Act.Ln, bias=eps_sb[:1, :])
            rstd_row = stats.tile([1, TOK], F32R, tag="rstd")
            nc.scalar.activation(out=rstd_row[:], in_=var_row[:],
                                 func=Act.Exp, scale=-0.5)
            return m_row, rstd_row

        # ---------------- per-block loop ----------------
        for i in range(NB):
            rev = (i % 2 == 1)
            wi_sb = wpool.tile([128, 64 * 128], F16, tag="wi")
            nc.sync.dma_start(out=wi_sb[:], in_=Tn["wi"].ap()[i])
            negrs_sb = wpool.tile([1, 16 * 128], F16, tag="negrs")
            nc.sync.dma_start(out=negrs_sb[:], in_=Tn["negrs"].ap()[i])
            lnb_sb = None
            if has_lnb:
                lnb_sb = wpool.tile([1, 16 * 128], F16, tag="lnb")
                nc.sync.dma_start(out=lnb_sb[:], in_=Tn["lnbias"].ap()[i])
            convd_sb = wpool.tile([128, KCONV * 128], F16, tag="convd")
            nc.sync.dma_start(out=convd_sb[:], in_=Tn["convd"].ap()[i])
            convb_sb = wpool.tile([128, 1], F32, tag="convb")
            nc.sync.dma_start(out=convb_sb[:], in_=Tn["convb"].ap()[i])
            wx_sb = wpool.tile([128, 64], F16, tag="wx")
            nc.sync.dma_start(out=wx_sb[:], in_=Tn["wx"].ap()[i])
            wdt_sb = wpool.tile([32, 128], F16, tag="wdt")
            nc.sync.dma_start(out=wdt_sb[:], in_=Tn["wdt"].ap()[i])
            bdt_sb = wpool.tile([1, 128], F16, tag="bdt")
            nc.sync.dma_start(out=bdt_sb[:], in_=Tn["bdt"].ap()[i])
            acols_sb = wpool.tile([128, NST], F32, tag="acols")
            nc.sync.dma_start(out=acols_sb[:], in_=Tn["acols"].ap()[i])
            dpcol_sb = wpool.tile([128, 1], F32, tag="dpcol")
            nc.sync.dma_start(out=dpcol_sb[:], in_=Tn["dpcol"].ap()[i])
            wo_sb = wpool.tile([128, 32 * 128], F16, tag="wo")
            nc.sync.dma_start(out=wo_sb[:], in_=Tn["wo"].ap()[i])

            # ---- T1: LN + in-proj (token-shard), xi groups then z ----
            m_row, rstd_row = ln_head(x_cur)
            mrs16 = stats.tile([1, TOK], F16, tag="mrs16")
            nc.vector.tensor_mul(out=mrs16[:], in0=m_row[:].bitcast(F32),
                                 in1=rstd_row[:].bitcast(F32))
            rbc = ps_bc.tile([128, TOK], F32, tag="bc", name="rbc")
            mm(rbc[:], lhsT=ones1[:], rhs=rstd_row[:], start=True, stop=True)
            xn = []
            for g in range(NG):
                xng = xnpool.tile([128, TOK], F16, tag=f"xn{g}",
                                  name=f"xn{g}")
                nc.vector.tensor_mul(out=xng[:],
                                     in0=x_cur[g][:].bitcast(F32),
                                     in1=rbc[:])
                xn.append(xng)

            def inproj_group(g, psname):
                xz = ps_mm.tile([128, TOK], F32, tag="mm", name=psname)
                for kc in range(NG):
                    lh = wi_sb[:, (g * 4 + kc) * 128:(g * 4 + kc + 1) * 128]
                    mm(xz[:], lhsT=lh, rhs=xn[kc][:],
                       start=(kc == 0), stop=False)
                mm(xz[:], lhsT=negrs_sb[:, g * 128:(g + 1) * 128],
                   rhs=mrs16[:], start=False, stop=(not has_lnb))
                if has_lnb:
                    mm(xz[:], lhsT=lnb_sb[:, g * 128:(g + 1) * 128],
                       rhs=ones16[:], start=False, stop=True)
                return xz

            for g in range(NDI):          # xi rows
                xz = inproj_group(g, f"xz_xi{g}")
                xi16 = evac.tile([128, TOK], F16, tag="xi16")
                nc.scalar.copy(out=xi16[:], in_=xz[:])
                nc.sync.dma_start(
                    out=Tn["xia_in"][i].ap()[128 * g:128 * (g + 1), :],
                    in_=xi16[:])
            nc.gpsimd.collective_compute(
                "AllToAll", Alu.bypass, replica_groups=RG,
                ins=[Tn["xia_in"][i].ap()], outs=[Tn["xia_out"][i].ap()])
            load_table(SILU_SET)
            zs = []
            for g in range(NDI):          # z rows -> silu, kept in SBUF
                xz = inproj_group(NDI + g, f"xz_z{g}")
                zg = zpool.tile([128, TOK], F16, tag=f"zs{g}", name=f"zs{g}")
                nc.scalar.activation(out=zg[:], in_=xz[:], func=Act.Silu)
                zs.append(zg)

            # ---- C1: conv + Wx + AR + dt + scan (channel-shard) ----
            xipad = [bigs.tile([128, L + 6], F16, tag=f"xipad{b}",
                               name=f"xipad{b}") for b in range(B)]
            xis = [bigs.tile([128, L], F16, tag=f"xis{b}", name=f"xis{b}")
                   for b in range(B)]
            for b in range(B):
                nc.vector.memset(xipad[b][:, 0:3], 0.0)
                nc.vector.memset(xipad[b][:, L + 3:L + 6], 0.0)
            for s in range(NCORES):
                b, q = s // NCH, s % NCH
                nc.sync.dma_start(
                    out=xipad[b][:, 3 + CH * q: 3 + CH * (q + 1)],
                    in_=Tn["xia_out"][i].ap()[128 * s:128 * (s + 1), :])

            for b in range(B):
                for c in range(NCH):
                    t0 = c * CH
                    cv = ps_mm.tile([128, CH], F32, tag="mm")
                    for kk in range(KCONV):
                        off = t0 + (kk if not rev else (6 - kk))
                        mm(cv[:], lhsT=convd_sb[:, kk * 128:(kk + 1) * 128],
                           rhs=xipad[b][:, off: off + CH],
                           start=(kk == 0), stop=(kk == KCONV - 1))
                    nc.scalar.activation(out=xis[b][:, t0:t0 + CH], in_=cv[:],
                                         func=Act.Silu, bias=convb_sb[:])
                    wxp = ps_mm.tile([64, CH], F32, tag="mm", name="wxp")
                    mm(wxp[:], lhsT=wx_sb[:], rhs=xis[b][:, t0:t0 + CH],
                       start=True, stop=True)
                    xd16c = evac.tile([64, CH], F16, tag="xd16c")
                    nc.scalar.copy(out=xd16c[:], in_=wxp[:])
                    nc.sync.dma_start(
                        out=Tn["xd_in"][i].ap()[:, b * L + t0:b * L + t0 + CH],
                        in_=xd16c[:])
            nc.gpsimd.collective_compute(
                "AllReduce", Alu.add, replica_groups=RG,
                ins=[Tn["xd_in"][i].ap()], outs=[Tn["xd_out"][i].ap()])

            dtr32 = bigs.tile([32, T], F16, tag="dtr32", name="dtr32")
            nc.sync.dma_start(out=dtr32[:], in_=Tn["xd_out"][i].ap()[0:32, :])

            load_table(LN_EXP_SET)
            dt = [bigs.tile([128, L], F16, tag=f"dt{b}", name=f"dt{b}")
                  for b in range(B)]
            dtx = [bigs.tile([128, L], F16, tag=f"dtx{b}", name=f"dtx{b}")
                   for b in range(B)]
            for b in range(B):
                for c in range(NCH):
                    t0 = c * CH
                    gt0 = b * L + t0
                    dt_ps = ps_mm.tile([128, CH], F32, tag="mm",
                                       name="dt_ps")
                    mm(dt_ps[:], lhsT=wdt_sb[:], rhs=dtr32[:, gt0:gt0 + CH],
                       start=True, stop=False)
                    mm(dt_ps[:], lhsT=bdt_sb[:], rhs=ones16[:],
                       start=False, stop=True)
                    e_sb = small.tile([128, CH], F16, tag="sp_e", name="e_sb")
                    nc.scalar.activation(out=e_sb[:], in_=dt_ps[:],
                                         func=Act.Exp, scale=-1.0)
                    nc.scalar.activation(out=e_sb[:], in_=e_sb[:],
                                         func=Act.Ln, bias=1.0)
                    dtr = small.tile([128, CH], F16, tag="dtr", name="dtr")
                    nc.scalar.copy(out=dtr[:], in_=dt_ps[:])
                    nc.vector.tensor_add(out=dt[b][:, t0:t0 + CH],
                                         in0=dtr[:], in1=e_sb[:])
                    nc.vector.tensor_mul(out=dtx[b][:, t0:t0 + CH],
                                         in0=dt[b][:, t0:t0 + CH],
                                         in1=xis[b][:, t0:t0 + CH])

            # ---- scan: full-batch per state; Pool scans, DVE muls ----
            for b in range(B):
                y_ps = [ps_y.tile([128, CH], F32, tag=f"y{c}", name=f"y{c}")
                        for c in range(NCH)]

                def revfull(tl):
                    return tl[:, L - 1::-1] if rev else tl[:, 0:L]

                def revchunk(tl, t0o):
                    if not rev:
                        return tl[:, t0o:t0o + CH]
                    if t0o == 0:
                        return tl[:, CH - 1::-1]
                    return tl[:, t0o + CH - 1:t0o - 1:-1]

                for n in range(NST):
                    pe_route = (n % 8 == 7)
                    dA = spool.tile([128, L], F16, tag="dA")
                    nc.scalar.activation(out=dA[:], in_=revfull(dt[b]),
                                         func=Act.Exp,
                                         scale=acols_sb[:, n:n + 1])
                    brow = rowpool.tile([1, L], F16, tag="brow")
                    nc.sync.dma_start(
                        out=brow[:],
                        in_=Tn["xd_out"][i].ap()[32 + n:33 + n,
                                                 b * L:(b + 1) * L])
                    dBu = spool.tile([128, L], F16, tag="dBu")
                    if pe_route:
                        for c in range(NCH):
                            co = (NCH - 1 - c) if rev else c
                            t0o = co * CH
                            bbc = ps_bc.tile([128, CH], F32, tag="bc")
                            mm(bbc[:], lhsT=ones1h[:],
                               rhs=brow[0:1, t0o:t0o + CH],
                               start=True, stop=True)
                            bbc_r = bbc[:, CH - 1::-1] if rev else bbc[:]
                            nc.vector.tensor_mul(
                                out=dBu[:, c * CH:(c + 1) * CH],
                                in0=revchunk(dtx[b], t0o), in1=bbc_r)
                    else:
                        bbc_sb = bcast.tile([128, L], F16, tag="bbc")
                        nc.gpsimd.partition_broadcast(bbc_sb[:], brow[0:1, :])
                        nc.vector.tensor_mul(out=dBu[:], in0=revfull(dtx[b]),
                                             in1=revfull(bbc_sb))
                    h = hpool.tile([128, L], F16, tag="h")
                    nc.vector.tensor_tensor_scan(h[:], dA[:], dBu[:], 0.0,
                                                 op0=Alu.mult, op1=Alu.add)
                    crow = rowpool.tile([1, L], F16, tag="crow")
                    nc.sync.dma_start(
                        out=crow[:],
                        in_=Tn["xd_out"][i].ap()[48 + n:49 + n,
                                                 b * L:(b + 1) * L])
                    yterm = ytpool.tile([128, L], F16, tag="yterm")
                    if pe_route:
                        for c in range(NCH):
                            co = (NCH - 1 - c) if rev else c
                            t0o = co * CH
                            cbc = ps_bc.tile([128, CH], F32, tag="bc")
                            mm(cbc[:], lhsT=ones1h[:],
                               rhs=crow[0:1, t0o:t0o + CH],
                               start=True, stop=True)
                            cbc_r = cbc[:, CH - 1::-1] if rev else cbc[:]
                            nc.vector.tensor_mul(
                                out=yterm[:, c * CH:(c + 1) * CH],
                                in0=h[:, c * CH:(c + 1) * CH], in1=cbc_r)
                    else:
                        cbc_sb = bcast.tile([128, L], F16, tag="cbc")
                        nc.gpsimd.partition_broadcast(cbc_sb[:], crow[0:1, :])
                        nc.vector.tensor_mul(out=yterm[:], in0=h[:],
                                             in1=revfull(cbc_sb))
                    for c in range(NCH):
                        mm(y_ps[c][:], lhsT=ident[:],
                           rhs=yterm[:, c * CH:(c + 1) * CH],
                           start=(n == 0), stop=(n == NST - 1))
                for c in range(NCH):
                    co = (NCH - 1 - c) if rev else c
                    t0o = co * CH
                    gc = NCH * b + co
                    y16 = evac.tile([128, CH], F16, tag="y16")
                    yout = y16[:, CH - 1::-1] if rev else y16[:]
                    nc.scalar.copy(out=yout, in_=y_ps[c][:])
                    yfin = evac.tile([128, CH], F16, tag="yfin")
                    nc.vector.scalar_tensor_tensor(
                        out=yfin[:], in0=xis[b][:, t0o:t0o + CH],
                        scalar=dpcol_sb[:], in1=y16[:],
                        op0=Alu.mult, op1=Alu.add)
                    nc.sync.dma_start(
                        out=Tn["y_in"][i].ap()[128 * gc:128 * (gc + 1), :],
                        in_=yfin[:])
            nc.gpsimd.collective_compute(
                "AllToAll", Alu.bypass, replica_groups=RG,
                ins=[Tn["y_in"][i].ap()], outs=[Tn["y_out"][i].ap()])

            # ---- T2: gate + out-proj (token-shard) ----
            yg = []
            for s in range(NDI):
                ydn = evac.tile([128, TOK], F16, tag="ydn")
                nc.sync.dma_start(
                    out=ydn[:],
                    in_=Tn["y_out"][i].ap()[128 * s:128 * (s + 1), :])
                ygs = ygpool.tile([128, TOK], F16, tag=f"yg{s}",
                                  name=f"yg{s}")
                nc.vector.tensor_mul(out=ygs[:], in0=ydn[:], in1=zs[s][:])
                yg.append(ygs)
            x_next = []
            for g in range(NG):
                op_ps = ps_mm.tile([128, TOK], F32, tag="mm")
                for kc in range(NDI):
                    lh = wo_sb[:, (g * 8 + kc) * 128:(g * 8 + kc + 1) * 128]
                    mm(op_ps[:], lhsT=lh, rhs=yg[kc][:],
                       start=(kc == 0), stop=(kc == NDI - 1))
                xg = xcur_p.tile([128, TOK], F32R, tag=f"xcur{g}",
                                 name=f"xcur{g}_{i}")
                nc.scalar.copy(out=xg[:], in_=op_ps[:])
                x_next.append(xg)
            x_cur = x_next

        # ---------------- final layernorm (token-shard) ----------------
        m_row, rstd_row = ln_head(x_cur)
        mbc = ps_bc.tile([128, TOK], F32, tag="bc", name="mbc")
        mm(mbc[:], lhsT=ones1[:], rhs=m_row[:], start=True, stop=True)
        rbc_ps = ps_bc.tile([128, TOK], F32, tag="bc", name="rbcf")
        mm(rbc_ps[:], lhsT=ones1[:], rhs=rstd_row[:], start=True, stop=True)
        rbc = small.tile([128, TOK], F32, tag="rbc")
        nc.scalar.copy(out=rbc[:], in_=rbc_ps[:])
        for g in range(NG):
            t1_sb = small.tile([128, TOK], F32, tag="xsq", name="t1_sb")
            nc.vector.tensor_sub(out=t1_sb[:],
                                 in0=x_cur[g][:].bitcast(F32), in1=mbc[:])
            o_sb = opool.tile([128, TOK], F32, tag="o_sb")
            nc.vector.scalar_tensor_tensor(
                out=o_sb[:], in0=t1_sb[:], scalar=nfw_sb[:, g:g + 1],
                in1=rbc[:], op0=Alu.mult, op1=Alu.mult)
            if has_nfb:
                nc.vector.tensor_scalar_add(
                    out=o_sb[:], in0=o_sb[:], scalar1=nfb_sb[:, g:g + 1])
            nc.sync.dma_start(
                out=Tn["outs"].ap()[128 * g:128 * (g + 1), :], in_=o_sb[:])


def _host_prep(inputs):
    x = np.asarray(inputs["x"], np.float32)
    ln_w = np.asarray(inputs["ln_w"], np.float32)
    ln_b = np.asarray(inputs["ln_b"], np.float32)
    W_in = np.asarray(inputs["W_in"], np.float32)
    conv_w = np.asarray(inputs["conv_w"], np.float32)
    conv_b = np.asarray(inputs["conv_b"], np.float32)
    W_x = np.asarray(inputs["W_x"], np.float32)
    W_dt = np.asarray(inputs["W_dt"], np.float32)
    b_dt = np.asarray(inputs["b_dt"], np.float32)
    A_log = np.asarray(inputs["A_log"], np.float32)
    D_p = np.asarray(inputs["D_p"], np.float32)
    W_out = np.asarray(inputs["W_out"], np.float32)
    normf_w = np.asarray(inputs["normf_w"], np.float32)
    normf_b = np.asarray(inputs["normf_b"], np.float32)

    xT = np.ascontiguousarray(x.transpose(2, 0, 1).reshape(D, T))
    A = -np.exp(A_log)

    wi_arr = np.zeros((NB, 128, 64 * 128), np.float16)
    negrs_arr = np.zeros((NB, 1, 16 * 128), np.float16)
    lnb_arr = np.zeros((NB, 1, 16 * 128), np.float16)
    wo_arr = np.zeros((NB, 128, 32 * 128), np.float16)
    for i in range(NB):
        Wf = W_in[i] * ln_w[i][None, :]
        for g in range(16):
            rows = slice(128 * g, 128 * (g + 1))
            for kc in range(4):
                cols = slice(128 * kc, 128 * (kc + 1))
                wi_arr[i, :, (g * 4 + kc) * 128:(g * 4 + kc + 1) * 128] = \
                    Wf[rows, cols].T
            negrs_arr[i, 0, g * 128:(g + 1) * 128] = -Wf[rows, :].sum(1)
            lnb_arr[i, 0, g * 128:(g + 1) * 128] = W_in[i][rows, :] @ ln_b[i]
        for g in range(4):
            rows = slice(128 * g, 128 * (g + 1))
            for kc in range(8):
                cols = slice(128 * kc, 128 * (kc + 1))
                wo_arr[i, :, (g * 8 + kc) * 128:(g * 8 + kc + 1) * 128] = \
                    W_out[i][rows, cols].T

    selbc = np.zeros((64, 32 * 128), np.float16)
    for q in range(32):
        selbc[32 + q, q * 128:(q + 1) * 128] = 1.0

    nfw = np.ascontiguousarray(normf_w.reshape(NG, 128).T)
    nfb = np.ascontiguousarray(normf_b.reshape(NG, 128).T)
    identin = np.eye(128, dtype=np.float16)

    in_maps = []
    for k in range(NCORES):
        sl = slice(128 * k, 128 * (k + 1))
        convd_arr = np.zeros((NB, 128, KCONV * 128), np.float16)
        convb_arr = np.zeros((NB, 128, 1), np.float32)
        wx_arr = np.zeros((NB, 128, 64), np.float16)
        wdt_arr = np.zeros((NB, 32, 128), np.float16)
        bdt_arr = np.zeros((NB, 1, 128), np.float16)
        acols_arr = np.zeros((NB, 128, NST), np.float32)
        dp_arr = np.zeros((NB, 128, 1), np.float32)
        for i in range(NB):
            for kk in range(KCONV):
                np.fill_diagonal(convd_arr[i, :, kk * 128:(kk + 1) * 128],
                                 conv_w[i, sl, kk])
            convb_arr[i, :, 0] = conv_b[i, sl]
            wx_arr[i] = W_x[i][:, sl].T
            wdt_arr[i] = W_dt[i][sl, :].T
            bdt_arr[i, 0, :] = b_dt[i, sl]
            acols_arr[i] = A[i, sl, :]
            dp_arr[i, :, 0] = D_p[i, sl]
        in_maps.append({
            "xs": np.ascontiguousarray(xT[:, TOK * k:TOK * (k + 1)]),
            "wi": wi_arr, "negrs": negrs_arr, "lnbias": lnb_arr,
            "convd": convd_arr, "convb": convb_arr,
            "wx": wx_arr, "wdt": wdt_arr, "bdt": bdt_arr,
            "acols": acols_arr, "dpcol": dp_arr, "wo": wo_arr,
            "nfw": nfw, "nfb": nfb, "identin": identin, "selbc": selbc,
        })
    has_lnb = bool(np.any(ln_b != 0.0))
    has_nfb = bool(np.any(normf_b != 0.0))
    return in_maps, has_lnb, has_nfb


def _get_program(has_lnb, has_nfb):
    key = (has_lnb, has_nfb)
    if key not in _PROGRAM_CACHE:
        _PROGRAM_CACHE[key] = _build_program(has_lnb, has_nfb)
    return _PROGRAM_CACHE[key]


def kernel(**inputs) -> np.ndarray:
    global _LAST_RESULTS
    in_maps, has_lnb, has_nfb = _host_prep(inputs)
    nc = _get_program(has_lnb, has_nfb)
    res = bass_utils.run_bass_kernel_spmd(nc, in_maps,
                                          core_ids=list(range(NCORES)))
    _LAST_RESULTS = res
    out_T = np.concatenate([res.results[k]["outs"] for k in range(NCORES)],
                           axis=1)
    out = out_T.reshape(D, B, L).transpose(1, 2, 0)
    return np.ascontiguousarray(out.astype(np.float32))


# revision 6
# speedup vs baseline: 1.0774x; 1.0614x over previous
"""Trainium2 Bass kernel for the bidirectional Mamba MixerModel (AllToAll dataflow).

Sharding alternates per block between token-sharding (each of 8 cores owns 512
global tokens, full model width: LayerNorm, in-proj, gating, out-proj) and
channel-sharding (each core owns 128 of 1024 d_inner channels, all 4096
tokens: conv, selective scan).  The layouts are bridged by two fp16 1MB
AllToAll transposes plus a fp16 ReduceScatter+AllGather for x_dbl per block —
replacing Megatron-style 4MB AllReduces, which dominated the baseline.

The inter-block sequence flip is pure relabeling: activations stay in original
token coordinates; odd blocks run conv taps shifted the other way and the scan
over reversed access patterns (no data movement, no extra collectives).

Scan engine placement: scans are full-batch [128, 2048] DVE ops (one per
state, no inter-chunk carry chain; fp32 recurrence state, fp16 operands).
B/C row broadcasts land in fp16 SBUF (so the dBu/yterm multiplies hit the
DVE 2-byte 2x mode) via two routes balanced across engines: GpSimd
partition_broadcast, and a PE rank-1 matmul evacuated by the Scalar engine.
Activation-table loads are emitted manually (2 per block: exp/ln <-> silu).

Weights ride in fp16 (in/out projections fully replicated per core for the
token-shard phases); verified end-to-end rel err ~3.1e-3 vs the fp32
reference (tolerance 2e-2).
"""
import sys
import numpy as np

sys.path.insert(0, "/opt/trn_rl_repo")

import concourse.bass as bass  # noqa: E402,F401
import concourse.bacc as bacc  # noqa: E402
import concourse.tile as tile  # noqa: E402
from concourse import mybir  # noqa: E402
from concourse import bass_utils  # noqa: E402

F32 = mybir.dt.float32
F32R = mybir.dt.float32r
F16 = mybir.dt.float16
Alu = mybir.AluOpType
Act = mybir.ActivationFunctionType

B, L, D, DI = 2, 2048, 512, 1024
NST, KCONV, RDT, NB = 16, 4, 32, 4
NCORES = 8
T = B * L
TOK = T // NCORES          # 512 tokens per core in token-shard phases
CH = 512
NCH = L // CH              # 4 chunks per batch
NG = D // 128
NDI = DI // 128
EPS = 1e-5
LN_EXP_SET = 6             # natural_log_exp_and_others in act_info.json
SILU_SET = 18              # silu_and_others

_PROGRAM_CACHE = {}
_LAST_RESULTS = None


def _build_program(has_lnb: bool, has_nfb: bool):
    nc = bacc.Bacc("TRN2", target_bir_lowering=False, debug=False,
                   enable_asserts=False, num_devices=NCORES)

    Tn = {}
    Tn["xs"] = nc.dram_tensor("xs", [D, TOK], F32, kind="ExternalInput")
    Tn["wi"] = nc.dram_tensor("wi", [NB, 128, 64 * 128], F16, kind="ExternalInput")
    Tn["negrs"] = nc.dram_tensor("negrs", [NB, 1, 16 * 128], F16, kind="ExternalInput")
    Tn["lnbias"] = nc.dram_tensor("lnbias", [NB, 1, 16 * 128], F16, kind="ExternalInput")
    Tn["convd"] = nc.dram_tensor("convd", [NB, 128, KCONV * 128], F16, kind="ExternalInput")
    Tn["convb"] = nc.dram_tensor("convb", [NB, 128, 1], F32, kind="ExternalInput")
    Tn["wx"] = nc.dram_tensor("wx", [NB, 128, 64], F16, kind="ExternalInput")
    Tn["wdt"] = nc.dram_tensor("wdt", [NB, 32, 128], F16, kind="ExternalInput")
    Tn["bdt"] = nc.dram_tensor("bdt", [NB, 1, 128], F16, kind="ExternalInput")
    Tn["acols"] = nc.dram_tensor("acols", [NB, 128, NST], F32, kind="ExternalInput")
    Tn["dpcol"] = nc.dram_tensor("dpcol", [NB, 128, 1], F32, kind="ExternalInput")
    Tn["wo"] = nc.dram_tensor("wo", [NB, 128, 32 * 128], F16, kind="ExternalInput")
    Tn["nfw"] = nc.dram_tensor("nfw", [128, NG], F32, kind="ExternalInput")
    Tn["nfb"] = nc.dram_tensor("nfb", [128, NG], F32, kind="ExternalInput")
    Tn["identin"] = nc.dram_tensor("identin", [128, 128], F16, kind="ExternalInput")
    Tn["selbc"] = nc.dram_tensor("selbc", [64, 32 * 128], F16, kind="ExternalInput")
    Tn["outs"] = nc.dram_tensor("outs", [D, TOK], F32, kind="ExternalOutput")

    xia_in, xia_out, xd_in, xd_out, y_in, y_out = [], [], [], [], [], []
    for i in range(NB):
        xia_in.append(nc.dram_tensor(f"xia_in_{i}", [DI, TOK], F16, kind="Internal"))
        xia_out.append(nc.dram_tensor(f"xia_out_{i}", [DI, TOK], F16, kind="Internal"))
        xd_in.append(nc.dram_tensor(f"xd_in_{i}", [64, T], F16, kind="Internal"))
        xd_out.append((nc.dram_tensor(f"xd_rs_{i}", [64, TOK], F16, kind="Internal"),
                       nc.dram_tensor(f"xd_ag_{i}", [NCORES, 64, TOK], F16, kind="Internal")))
        y_in.append(nc.dram_tensor(f"y_in_{i}", [DI, TOK], F16, kind="Internal"))
        y_out.append(nc.dram_tensor(f"y_out_{i}", [DI, TOK], F16, kind="Internal"))
    Tn["xia_in"], Tn["xia_out"] = xia_in, xia_out
    Tn["xd_in"], Tn["xd_out"] = xd_in, xd_out
    Tn["y_in"], Tn["y_out"] = y_in, y_out

    with tile.TileContext(nc) as tc:
        _emit(nc, tc, Tn, has_lnb, has_nfb)

    nc.compile()
    return nc


def _emit(nc, tc, Tn, has_lnb, has_nfb):
    import contextlib
    RG = [list(range(NCORES))]

    def load_table(set_id):
        nc.scalar.add_instruction(mybir.InstLoadActFuncSet(
            name=nc.get_next_instruction_name(), ins=[], outs=[],
            act_func_set_id=set_id))

    ctx = contextlib.ExitStack()
    with ctx:
        consts = ctx.enter_context(tc.tile_pool(name="consts", bufs=1))
        wpool = ctx.enter_context(tc.tile_pool(name="wpool", bufs=1))
        small = ctx.enter_context(tc.tile_pool(name="small", bufs=2))
        xnpool = ctx.enter_context(tc.tile_pool(name="xnpool", bufs=1))
        ygpool = ctx.enter_context(tc.tile_pool(name="ygpool", bufs=1))
        stats = ctx.enter_context(tc.tile_pool(name="stats", bufs=1))
        bigs = ctx.enter_context(tc.tile_pool(name="bigs", bufs=1))
        zpool = ctx.enter_context(tc.tile_pool(name="zpool", bufs=1))
        xcur_p = ctx.enter_context(tc.tile_pool(name="xcur", bufs=1))
        spool = ctx.enter_context(tc.tile_pool(name="spool", bufs=2))
        hpool = ctx.enter_context(tc.tile_pool(name="hpool", bufs=1))
        ytpool = ctx.enter_context(tc.tile_pool(name="ytpool", bufs=1))
        opool = ctx.enter_context(tc.tile_pool(name="opool", bufs=1))
        rowpool = ctx.enter_context(tc.tile_pool(name="rowpool", bufs=2))
        bcast = ctx.enter_context(tc.tile_pool(name="bcast", bufs=3))
        evac = ctx.enter_context(tc.tile_pool(name="evac", bufs=2))
        ps_mm = ctx.enter_context(tc.tile_pool(name="ps_mm", bufs=2, space="PSUM"))
        ps_bc = ctx.enter_context(tc.tile_pool(name="ps_bc", bufs=2, space="PSUM"))
        ps_y = ctx.enter_context(tc.tile_pool(name="ps_y", bufs=1, space="PSUM"))

        def mm(out, lhsT, rhs, **kw):
            nc.tensor.matmul(out, lhsT=lhsT, rhs=rhs, **kw)

        # ------------- constants -------------
        ident = consts.tile([128, 128], F16, tag="ident")
        nc.sync.dma_start(out=ident[:], in_=Tn["identin"].ap())
        onescol = consts.tile([128, 1], F32R, tag="onescol")
        nc.vector.memset(onescol[:].bitcast(F32), 1.0)
        ones1 = consts.tile([1, 128], F32R, tag="ones1")
        nc.vector.memset(ones1[:].bitcast(F32), 1.0)
        ones16 = consts.tile([1, CH], F16, tag="ones16")
        nc.vector.memset(ones16[:], 1.0)
        ones1h = consts.tile([1, 128], F16, tag="ones1h")
        nc.vector.memset(ones1h[:], 1.0)
        eps_sb = consts.tile([128, 1], F32, tag="eps")
        nc.vector.memset(eps_sb[:], EPS)
        nfw_sb = consts.tile([128, NG], F32, tag="nfw")
        nc.sync.dma_start(out=nfw_sb[:], in_=Tn["nfw"].ap())
        nfb_sb = consts.tile([128, NG], F32, tag="nfb")
        nc.sync.dma_start(out=nfb_sb[:], in_=Tn["nfb"].ap())

        load_table(LN_EXP_SET)

        def load_x0():
            xt = []
            for g in range(NG):
                xg = xcur_p.tile([128, TOK], F32R, tag=f"xcur{g}",
                                 name=f"xcur{g}_init")
                nc.sync.dma_start(
                    out=xg[:],
                    in_=Tn["xs"].ap()[128 * g:128 * (g + 1), :].bitcast(F32R))
                xt.append(xg)
            return xt

        x_cur = load_x0()

        def ln_head(x_tiles):
            """stats -> (m_row f32r, rstd_row f32r, mrs16 f16)"""
            s1 = ps_bc.tile([1, TOK], F32, tag="bc", name="s1")
            s2 = ps_bc.tile([1, TOK], F32, tag="bc", name="s2")
            for g in range(NG):
                xsq = small.tile([128, TOK], F32R, tag="xsq")
                nc.scalar.square(out=xsq[:], in_=x_tiles[g][:].bitcast(F32))
                mm(s1[:], lhsT=onescol[:], rhs=x_tiles[g][:],
                   start=(g == 0), stop=(g == NG - 1))
                mm(s2[:], lhsT=onescol[:], rhs=xsq[:],
                   start=(g == 0), stop=(g == NG - 1))
            m_row = stats.tile([1, TOK], F32R, tag="mrow")
            nc.vector.tensor_scalar_mul(out=m_row[:], in0=s1[:],
                                        scalar1=1.0 / D)
            mu2 = small.tile([1, TOK], F32, tag="mu2")
            nc.vector.tensor_mul(out=mu2[:], in0=m_row[:].bitcast(F32),
                                 in1=m_row[:].bitcast(F32))
            var_row = stats.tile([1, TOK], F32, tag="var")
            nc.vector.scalar_tensor_tensor(
                out=var_row[:], in0=s2[:], scalar=1.0 / D, in1=mu2[:],
                op0=Alu.mult, op1=Alu.subtract)
            nc.scalar.activation(out=var_row[:], in_=var_row[:],
                                 func=Act.Ln, bias=eps_sb[:1, :])
            rstd_row = stats.tile([1, TOK], F32R, tag="rstd")
            nc.scalar.activation(out=rstd_row[:], in_=var_row[:],
                                 func=Act.Exp, scale=-0.5)
            return m_row, rstd_row

        # ---------------- per-block loop ----------------
        for i in range(NB):
            rev = (i % 2 == 1)
            wi_sb = wpool.tile([128, 64 * 128], F16, tag="wi")
            nc.sync.dma_start(out=wi_sb[:], in_=Tn["wi"].ap()[i])
            negrs_sb = wpool.tile([1, 16 * 128], F16, tag="negrs")
            nc.sync.dma_start(out=negrs_sb[:], in_=Tn["negrs"].ap()[i])
            lnb_sb = None
            if has_lnb:
                lnb_sb = wpool.tile([1, 16 * 128], F16, tag="lnb")
                nc.sync.dma_start(out=lnb_sb[:], in_=Tn["lnbias"].ap()[i])
            convd_sb = wpool.tile([128, KCONV * 128], F16, tag="convd")
            nc.sync.dma_start(out=convd_sb[:], in_=Tn["convd"].ap()[i])
            convb_sb = wpool.tile([128, 1], F32, tag="convb")
            nc.sync.dma_start(out=convb_sb[:], in_=Tn["convb"].ap()[i])
            wx_sb = wpool.tile([128, 64], F16, tag="wx")
            nc.sync.dma_start(out=wx_sb[:], in_=Tn["wx"].ap()[i])
            wdt_sb = wpool.tile([32, 128], F16, tag="wdt")
            nc.sync.dma_start(out=wdt_sb[:], in_=Tn["wdt"].ap()[i])
            bdt_sb = wpool.tile([1, 128], F16, tag="bdt")
            nc.sync.dma_start(out=bdt_sb[:], in_=Tn["bdt"].ap()[i])
            acols_sb = wpool.tile([128, NST], F32, tag="acols")
            nc.sync.dma_start(out=acols_sb[:], in_=Tn["acols"].ap()[i])
            dpcol_sb = wpool.tile([128, 1], F32, tag="dpcol")
            nc.sync.dma_start(out=dpcol_sb[:], in_=Tn["dpcol"].ap()[i])
            wo_sb = wpool.tile([128, 32 * 128], F16, tag="wo")
            nc.sync.dma_start(out=wo_sb[:], in_=Tn["wo"].ap()[i])

            # ---- T1: LN + in-proj (token-shard), xi groups then z ----
            m_row, rstd_row = ln_head(x_cur)
            mrs16 = stats.tile([1, TOK], F16, tag="mrs16")
            nc.vector.tensor_mul(out=mrs16[:], in0=m_row[:].bitcast(F32),
                                 in1=rstd_row[:].bitcast(F32))
            rbc = ps_bc.tile([128, TOK], F32, tag="bc", name="rbc")
            mm(rbc[:], lhsT=ones1[:], rhs=rstd_row[:], start=True, stop=True)
            xn = []
            for g in range(NG):
                xng = xnpool.tile([128, TOK], F16, tag=f"xn{g}",
                                  name=f"xn{g}")
                nc.vector.tensor_mul(out=xng[:],
                                     in0=x_cur[g][:].bitcast(F32),
                                     in1=rbc[:])
                xn.append(xng)

            def inproj_group(g, psname):
                xz = ps_mm.tile([128, TOK], F32, tag="mm", name=psname)
                for kc in range(NG):
                    lh = wi_sb[:, (g * 4 + kc) * 128:(g * 4 + kc + 1) * 128]
                    mm(xz[:], lhsT=lh, rhs=xn[kc][:],
                       start=(kc == 0), stop=False)
                mm(xz[:], lhsT=negrs_sb[:, g * 128:(g + 1) * 128],
                   rhs=mrs16[:], start=False, stop=(not has_lnb))
                if has_lnb:
                    mm(xz[:], lhsT=lnb_sb[:, g * 128:(g + 1) * 128],
                       rhs=ones16[:], start=False, stop=True)
                return xz

            for g in range(NDI):          # xi rows
                xz = inproj_group(g, f"xz_xi{g}")
                xi16 = evac.tile([128, TOK], F16, tag="xi16")
                nc.scalar.copy(out=xi16[:], in_=xz[:])
                nc.sync.dma_start(
                    out=Tn["xia_in"][i].ap()[128 * g:128 * (g + 1), :],
                    in_=xi16[:])
            nc.gpsimd.collective_compute(
                "AllToAll", Alu.bypass, replica_groups=RG,
                ins=[Tn["xia_in"][i].ap()], outs=[Tn["xia_out"][i].ap()])
            load_table(SILU_SET)
            zs = []
            for g in range(NDI):          # z rows -> silu, kept in SBUF
                xz = inproj_group(NDI + g, f"xz_z{g}")
                zg = zpool.tile([128, TOK], F16, tag=f"zs{g}", name=f"zs{g}")
                nc.scalar.activation(out=zg[:], in_=xz[:], func=Act.Silu)
                zs.append(zg)

            # ---- C1: conv + Wx + AR + dt + scan (channel-shard) ----
            xipad = [bigs.tile([128, L + 6], F16, tag=f"xipad{b}",
                               name=f"xipad{b}") for b in range(B)]
            xis = [bigs.tile([128, L], F16, tag=f"xis{b}", name=f"xis{b}")
                   for b in range(B)]
            for b in range(B):
                nc.vector.memset(xipad[b][:, 0:3], 0.0)
                nc.vector.memset(xipad[b][:, L + 3:L + 6], 0.0)
            for s in range(NCORES):
                b, q = s // NCH, s % NCH
                nc.sync.dma_start(
                    out=xipad[b][:, 3 + CH * q: 3 + CH * (q + 1)],
                    in_=Tn["xia_out"][i].ap()[128 * s:128 * (s + 1), :])

            for b in range(B):
                for c in range(NCH):
                    t0 = c * CH
                    cv = ps_mm.tile([128, CH], F32, tag="mm")
                    for kk in range(KCONV):
                        off = t0 + (kk if not rev else (6 - kk))
                        mm(cv[:], lhsT=convd_sb[:, kk * 128:(kk + 1) * 128],
                           rhs=xipad[b][:, off: off + CH],
                           start=(kk == 0), stop=(kk == KCONV - 1))
                    nc.scalar.activation(out=xis[b][:, t0:t0 + CH], in_=cv[:],
                                         func=Act.Silu, bias=convb_sb[:])
                    wxp = ps_mm.tile([64, CH], F32, tag="mm", name="wxp")
                    mm(wxp[:], lhsT=wx_sb[:], rhs=xis[b][:, t0:t0 + CH],
                       start=True, stop=True)
                    xd16c = evac.tile([64, CH], F16, tag="xd16c")
                    nc.scalar.copy(out=xd16c[:], in_=wxp[:])
                    nc.sync.dma_start(
                        out=Tn["xd_in"][i].ap()[:, b * L + t0:b * L + t0 + CH],
                        in_=xd16c[:])
            xd_rs, xd_ag = Tn["xd_out"][i]
            nc.gpsimd.collective_compute(
                "ReduceScatter", Alu.add, replica_groups=RG,
                ins=[Tn["xd_in"][i].ap()], outs=[xd_rs.ap()])
            nc.gpsimd.collective_compute(
                "AllGather", Alu.bypass, replica_groups=RG,
                ins=[xd_rs.ap()], outs=[xd_ag.ap()])

            dtr32 = bigs.tile([32, T], F16, tag="dtr32", name="dtr32")
            for sl in range(NCORES):
                nc.sync.dma_start(
                    out=dtr32[:, TOK * sl:TOK * (sl + 1)],
                    in_=xd_ag.ap()[sl, 0:32, :])

            load_table(LN_EXP_SET)
            dt = [bigs.tile([128, L], F16, tag=f"dt{b}", name=f"dt{b}")
                  for b in range(B)]
            dtx = [bigs.tile([128, L], F16, tag=f"dtx{b}", name=f"dtx{b}")
                   for b in range(B)]
            for b in range(B):
                for c in range(NCH):
                    t0 = c * CH
                    gt0 = b * L + t0
                    dt_ps = ps_mm.tile([128, CH], F32, tag="mm",
                                       name="dt_ps")
                    mm(dt_ps[:], lhsT=wdt_sb[:], rhs=dtr32[:, gt0:gt0 + CH],
                       start=True, stop=False)
                    mm(dt_ps[:], lhsT=bdt_sb[:], rhs=ones16[:],
                       start=False, stop=True)
                    e_sb = small.tile([128, CH], F16, tag="sp_e", name="e_sb")
                    nc.scalar.activation(out=e_sb[:], in_=dt_ps[:],
                                         func=Act.Exp, scale=-1.0)
                    nc.scalar.activation(out=e_sb[:], in_=e_sb[:],
                                         func=Act.Ln, bias=1.0)
                    dtr = small.tile([128, CH], F16, tag="dtr", name="dtr")
                    nc.scalar.copy(out=dtr[:], in_=dt_ps[:])
                    nc.vector.tensor_add(out=dt[b][:, t0:t0 + CH],
                                         in0=dtr[:], in1=e_sb[:])
                    nc.vector.tensor_mul(out=dtx[b][:, t0:t0 + CH],
                                         in0=dt[b][:, t0:t0 + CH],
                                         in1=xis[b][:, t0:t0 + CH])

            # ---- scan: full-batch per state; Pool scans, DVE muls ----
            for b in range(B):
                y_ps = [ps_y.tile([128, CH], F32, tag=f"y{c}", name=f"y{c}")
                        for c in range(NCH)]

                def revfull(tl):
                    return tl[:, L - 1::-1] if rev else tl[:, 0:L]

                def revchunk(tl, t0o):
                    if not rev:
                        return tl[:, t0o:t0o + CH]
                    if t0o == 0:
                        return tl[:, CH - 1::-1]
                    return tl[:, t0o + CH - 1:t0o - 1:-1]

                for n in range(NST):
                    act_route = (n >= 8)     # C-side via PE rank-1 + Act evac
                    dA = spool.tile([128, L], F16, tag="dA")
                    nc.scalar.activation(out=dA[:], in_=revfull(dt[b]),
                                         func=Act.Exp,
                                         scale=acols_sb[:, n:n + 1])
                    brow = rowpool.tile([1, L], F16, tag="brow")
                    nc.sync.dma_start(
                        out=brow[:],
                        in_=Tn["xd_out"][i][1].ap()[
                            NCH * b:NCH * (b + 1), 32 + n:33 + n, :])
                    dBu = spool.tile([128, L], F16, tag="dBu")
                    bbc_sb = bcast.tile([128, L], F16, tag="bbc")
                    nc.gpsimd.partition_broadcast(bbc_sb[:], brow[0:1, :])
                    nc.vector.tensor_mul(out=dBu[:], in0=revfull(dtx[b]),
                                         in1=revfull(bbc_sb))
                    h = hpool.tile([128, L], F16, tag="h")
                    nc.vector.tensor_tensor_scan(h[:], dA[:], dBu[:], 0.0,
                                                 op0=Alu.mult, op1=Alu.add)
                    crow = rowpool.tile([1, L], F16, tag="crow")
                    nc.sync.dma_start(
                        out=crow[:],
                        in_=Tn["xd_out"][i][1].ap()[
                            NCH * b:NCH * (b + 1), 48 + n:49 + n, :])
                    cbc_sb = bcast.tile([128, L], F16, tag="cbc")
                    if act_route:
                        for c in range(NCH):
                            cbc = ps_bc.tile([128, CH], F32, tag="bc")
                            mm(cbc[:], lhsT=ones1h[:],
                               rhs=crow[0:1, c * CH:(c + 1) * CH],
                               start=True, stop=True)
                            nc.scalar.copy(
                                out=cbc_sb[:, c * CH:(c + 1) * CH],
                                in_=cbc[:])
                    else:
                        nc.gpsimd.partition_broadcast(cbc_sb[:], crow[0:1, :])
                    yterm = ytpool.tile([128, L], F16, tag="yterm")
                    nc.vector.tensor_mul(out=yterm[:], in0=h[:],
                                         in1=revfull(cbc_sb))
                    for c in range(NCH):
                        mm(y_ps[c][:], lhsT=ident[:],
                           rhs=yterm[:, c * CH:(c + 1) * CH],
                           start=(n == 0), stop=(n == NST - 1))
                for c in range(NCH):
                    co = (NCH - 1 - c) if rev else c
                    t0o = co * CH
                    gc = NCH * b + co
                    y16 = evac.tile([128, CH], F16, tag="y16")
                    yout = y16[:, CH - 1::-1] if rev else y16[:]
                    nc.scalar.copy(out=yout, in_=y_ps[c][:])
                    yfin = evac.tile([128, CH], F16, tag="yfin")
                    nc.vector.scalar_tensor_tensor(
                        out=yfin[:], in0=xis[b][:, t0o:t0o + CH],
                        scalar=dpcol_sb[:], in1=y16[:],
                        op0=Alu.mult, op1=Alu.add)
                    nc.sync.dma_start(
                        out=Tn["y_in"][i].ap()[128 * gc:128 * (gc + 1), :],
                        in_=yfin[:])
            nc.gpsimd.collective_compute(
                "AllToAll", Alu.bypass, replica_groups=RG,
                ins=[Tn["y_in"][i].ap()], outs=[Tn["y_out"][i].ap()])

            # ---- T2: gate + out-proj (token-shard) ----
            yg = []
            for s in range(NDI):
                ydn = evac.tile([128, TOK], F16, tag="ydn")
                nc.sync.dma_start(
                    out=ydn[:],
                    in_=Tn["y_out"][i].ap()[128 * s:128 * (s + 1), :])
                ygs = ygpool.tile([128, TOK], F16, tag=f"yg{s}",
                                  name=f"yg{s}")
                nc.vector.tensor_mul(out=ygs[:], in0=ydn[:], in1=zs[s][:])
                yg.append(ygs)
            x_next = []
            for g in range(NG):
                op_ps = ps_mm.tile([128, TOK], F32, tag="mm")
                for kc in range(NDI):
                    lh = wo_sb[:, (g * 8 + kc) * 128:(g * 8 + kc + 1) * 128]
                    mm(op_ps[:], lhsT=lh, rhs=yg[kc][:],
                       start=(kc == 0), stop=(kc == NDI - 1))
                xg = xcur_p.tile([128, TOK], F32R, tag=f"xcur{g}",
                                 name=f"xcur{g}_{i}")
                nc.scalar.copy(out=xg[:], in_=op_ps[:])
                x_next.append(xg)
            x_cur = x_next

        # ---------------- final layernorm (token-shard) ----------------
        m_row, rstd_row = ln_head(x_cur)
        mbc = ps_bc.tile([128, TOK], F32, tag="bc", name="mbc")
        mm(mbc[:], lhsT=ones1[:], rhs=m_row[:], start=True, stop=True)
        rbc_ps = ps_bc.tile([128, TOK], F32, tag="bc", name="rbcf")
        mm(rbc_ps[:], lhsT=ones1[:], rhs=rstd_row[:], start=True, stop=True)
        rbc = small.tile([128, TOK], F32, tag="rbc")
        nc.scalar.copy(out=rbc[:], in_=rbc_ps[:])
        for g in range(NG):
            t1_sb = small.tile([128, TOK], F32, tag="xsq", name="t1_sb")
            nc.vector.tensor_sub(out=t1_sb[:],
                                 in0=x_cur[g][:].bitcast(F32), in1=mbc[:])
            o_sb = opool.tile([128, TOK], F32, tag="o_sb")
            nc.vector.scalar_tensor_tensor(
                out=o_sb[:], in0=t1_sb[:], scalar=nfw_sb[:, g:g + 1],
                in1=rbc[:], op0=Alu.mult, op1=Alu.mult)
            if has_nfb:
                nc.vector.tensor_scalar_add(
                    out=o_sb[:], in0=o_sb[:], scalar1=nfb_sb[:, g:g + 1])
            nc.sync.dma_start(
                out=Tn["outs"].ap()[128 * g:128 * (g + 1), :], in_=o_sb[:])


def _host_prep(inputs):
    x = np.asarray(inputs["x"], np.float32)
    ln_w = np.asarray(inputs["ln_w"], np.float32)
    ln_b = np.asarray(inputs["ln_b"], np.float32)
    W_in = np.asarray(inputs["W_in"], np.float32)
    conv_w = np.asarray(inputs["conv_w"], np.float32)
    conv_b = np.asarray(inputs["conv_b"], np.float32)
    W_x = np.asarray(inputs["W_x"], np.float32)
    W_dt = np.asarray(inputs["W_dt"], np.float32)
    b_dt = np.asarray(inputs["b_dt"], np.float32)
    A_log = np.asarray(inputs["A_log"], np.float32)
    D_p = np.asarray(inputs["D_p"], np.float32)
    W_out = np.asarray(inputs["W_out"], np.float32)
    normf_w = np.asarray(inputs["normf_w"], np.float32)
    normf_b = np.asarray(inputs["normf_b"], np.float32)

    xT = np.ascontiguousarray(x.transpose(2, 0, 1).reshape(D, T))
    A = -np.exp(A_log)

    wi_arr = np.zeros((NB, 128, 64 * 128), np.float16)
    negrs_arr = np.zeros((NB, 1, 16 * 128), np.float16)
    lnb_arr = np.zeros((NB, 1, 16 * 128), np.float16)
    wo_arr = np.zeros((NB, 128, 32 * 128), np.float16)
    for i in range(NB):
        Wf = W_in[i] * ln_w[i][None, :]
        for g in range(16):
            rows = slice(128 * g, 128 * (g + 1))
            for kc in range(4):
                cols = slice(128 * kc, 128 * (kc + 1))
                wi_arr[i, :, (g * 4 + kc) * 128:(g * 4 + kc + 1) * 128] = \
                    Wf[rows, cols].T
            negrs_arr[i, 0, g * 128:(g + 1) * 128] = -Wf[rows, :].sum(1)
            lnb_arr[i, 0, g * 128:(g + 1) * 128] = W_in[i][rows, :] @ ln_b[i]
        for g in range(4):
            rows = slice(128 * g, 128 * (g + 1))
            for kc in range(8):
                cols = slice(128 * kc, 128 * (kc + 1))
                wo_arr[i, :, (g * 8 + kc) * 128:(g * 8 + kc + 1) * 128] = \
                    W_out[i][rows, cols].T

    selbc = np.zeros((64, 32 * 128), np.float16)
    for q in range(32):
        selbc[32 + q, q * 128:(q + 1) * 128] = 1.0

    nfw = np.ascontiguousarray(normf_w.reshape(NG, 128).T)
    nfb = np.ascontiguousarray(normf_b.reshape(NG, 128).T)
    identin = np.eye(128, dtype=np.float16)

    in_maps = []
    for k in range(NCORES):
        sl = slice(128 * k, 128 * (k + 1))
        convd_arr = np.zeros((NB, 128, KCONV * 128), np.float16)
        convb_arr = np.zeros((NB, 128, 1), np.float32)
        wx_arr = np.zeros((NB, 128, 64), np.float16)
        wdt_arr = np.zeros((NB, 32, 128), np.float16)
        bdt_arr = np.zeros((NB, 1, 128), np.float16)
        acols_arr = np.zeros((NB, 128, NST), np.float32)
        dp_arr = np.zeros((NB, 128, 1), np.float32)
        for i in range(NB):
            for kk in range(KCONV):
                np.fill_diagonal(convd_arr[i, :, kk * 128:(kk + 1) * 128],
                                 conv_w[i, sl, kk])
            convb_arr[i, :, 0] = conv_b[i, sl]
            wx_arr[i] = W_x[i][:, sl].T
            wdt_arr[i] = W_dt[i][sl, :].T
            bdt_arr[i, 0, :] = b_dt[i, sl]
            acols_arr[i] = A[i, sl, :]
            dp_arr[i, :, 0] = D_p[i, sl]
        in_maps.append({
            "xs": np.ascontiguousarray(xT[:, TOK * k:TOK * (k + 1)]),
            "wi": wi_arr, "negrs": negrs_arr, "lnbias": lnb_arr,
            "convd": convd_arr, "convb": convb_arr,
            "wx": wx_arr, "wdt": wdt_arr, "bdt": bdt_arr,
            "acols": acols_arr, "dpcol": dp_arr, "wo": wo_arr,
            "nfw": nfw, "nfb": nfb, "identin": identin, "selbc": selbc,
        })
    has_lnb = bool(np.any(ln_b != 0.0))
    has_nfb = bool(np.any(normf_b != 0.0))
    return in_maps, has_lnb, has_nfb


def _get_program(has_lnb, has_nfb):
    key = (has_lnb, has_nfb)
    if key not in _PROGRAM_CACHE:
        _PROGRAM_CACHE[key] = _build_program(has_lnb, has_nfb)
    return _PROGRAM_CACHE[key]


def kernel(**inputs) -> np.ndarray:
    global _LAST_RESULTS
    in_maps, has_lnb, has_nfb = _host_prep(inputs)
    nc = _get_program(has_lnb, has_nfb)
    res = bass_utils.run_bass_kernel_spmd(nc, in_maps,
                                          core_ids=list(range(NCORES)))
    _LAST_RESULTS = res
    out_T = np.concatenate([res.results[k]["outs"] for k in range(NCORES)],
                           axis=1)
    out = out_T.reshape(D, B, L).transpose(1, 2, 0)
    return np.ascontiguousarray(out.astype(np.float32))


# revision 7
# speedup vs baseline: 1.0774x; 1.0001x over previous
"""Trainium2 Bass kernel for the bidirectional Mamba MixerModel (AllToAll dataflow).

Sharding alternates per block between token-sharding (each of 8 cores owns 512
global tokens, full model width: LayerNorm, in-proj, gating, out-proj) and
channel-sharding (each core owns 128 of 1024 d_inner channels, all 4096
tokens: conv, selective scan).  The layouts are bridged by two fp16 1MB
AllToAll transposes plus a fp16 ReduceScatter+AllGather for x_dbl per block —
replacing Megatron-style 4MB AllReduces, which dominated the baseline.

The inter-block sequence flip is pure relabeling: activations stay in original
token coordinates; odd blocks run conv taps shifted the other way and the scan
over reversed access patterns (no data movement, no extra collectives).

Scan engine placement: scans are full-batch [128, 2048] DVE ops (one per
state, no inter-chunk carry chain; fp32 recurrence state, fp16 operands).
B/C row broadcasts land in fp16 SBUF (so the dBu/yterm multiplies hit the
DVE 2-byte 2x mode) via two routes balanced across engines: GpSimd
partition_broadcast, and a PE rank-1 matmul evacuated by the Scalar engine.
Activation-table loads are emitted manually (2 per block: exp/ln <-> silu).

Weights ride in fp16 (in/out projections fully replicated per core for the
token-shard phases); verified end-to-end rel err ~3.1e-3 vs the fp32
reference (tolerance 2e-2).
"""
import sys
import numpy as np

sys.path.insert(0, "/opt/trn_rl_repo")

import concourse.bass as bass  # noqa: E402,F401
import concourse.bacc as bacc  # noqa: E402
import concourse.tile as tile  # noqa: E402
from concourse import mybir  # noqa: E402
from concourse import bass_utils  # noqa: E402

F32 = mybir.dt.float32
F32R = mybir.dt.float32r
F16 = mybir.dt.float16
Alu = mybir.AluOpType
Act = mybir.ActivationFunctionType

B, L, D, DI = 2, 2048, 512, 1024
NST, KCONV, RDT, NB = 16, 4, 32, 4
NCORES = 8
T = B * L
TOK = T // NCORES          # 512 tokens per core in token-shard phases
CH = 512
NCH = L // CH              # 4 chunks per batch
NG = D // 128
NDI = DI // 128
EPS = 1e-5
LN_EXP_SET = 6             # natural_log_exp_and_others in act_info.json
SILU_SET = 18              # silu_and_others

_PROGRAM_CACHE = {}
_LAST_RESULTS = None


def _build_program(has_lnb: bool, has_nfb: bool):
    nc = bacc.Bacc("TRN2", target_bir_lowering=False, debug=False,
                   enable_asserts=False, num_devices=NCORES)

    Tn = {}
    Tn["xs"] = nc.dram_tensor("xs", [D, TOK], F32, kind="ExternalInput")
    Tn["wi"] = nc.dram_tensor("wi", [NB, 128, 64 * 128], F16, kind="ExternalInput")
    Tn["negrs"] = nc.dram_tensor("negrs", [NB, 1, 16 * 128], F16, kind="ExternalInput")
    Tn["lnbias"] = nc.dram_tensor("lnbias", [NB, 1, 16 * 128], F16, kind="ExternalInput")
    Tn["convd"] = nc.dram_tensor("convd", [NB, 128, KCONV * 128], F16, kind="ExternalInput")
    Tn["convb"] = nc.dram_tensor("convb", [NB, 128, 1], F32, kind="ExternalInput")
    Tn["wx"] = nc.dram_tensor("wx", [NB, 128, 64], F16, kind="ExternalInput")
    Tn["wdt"] = nc.dram_tensor("wdt", [NB, 32, 128], F16, kind="ExternalInput")
    Tn["bdt"] = nc.dram_tensor("bdt", [NB, 1, 128], F16, kind="ExternalInput")
    Tn["acols"] = nc.dram_tensor("acols", [NB, 128, NST], F32, kind="ExternalInput")
    Tn["dpcol"] = nc.dram_tensor("dpcol", [NB, 128, 1], F32, kind="ExternalInput")
    Tn["wo"] = nc.dram_tensor("wo", [NB, 128, 32 * 128], F16, kind="ExternalInput")
    Tn["nfw"] = nc.dram_tensor("nfw", [128, NG], F32, kind="ExternalInput")
    Tn["nfb"] = nc.dram_tensor("nfb", [128, NG], F32, kind="ExternalInput")
    Tn["identin"] = nc.dram_tensor("identin", [128, 128], F16, kind="ExternalInput")
    Tn["selbc"] = nc.dram_tensor("selbc", [64, 32 * 128], F16, kind="ExternalInput")
    Tn["outs"] = nc.dram_tensor("outs", [D, TOK], F32, kind="ExternalOutput")

    xia_in, xia_out, xd_in, xd_out, y_in, y_out = [], [], [], [], [], []
    for i in range(NB):
        xia_in.append(nc.dram_tensor(f"xia_in_{i}", [DI, TOK], F16, kind="Internal"))
        xia_out.append(nc.dram_tensor(f"xia_out_{i}", [DI, TOK], F16, kind="Internal"))
        xd_in.append(nc.dram_tensor(f"xd_in_{i}", [64, T], F16, kind="Internal"))
        xd_out.append((nc.dram_tensor(f"xd_rs_{i}", [64, TOK], F16, kind="Internal"),
                       nc.dram_tensor(f"xd_ag_{i}", [NCORES, 64, TOK], F16, kind="Internal")))
        y_in.append(nc.dram_tensor(f"y_in_{i}", [DI, TOK], F16, kind="Internal"))
        y_out.append(nc.dram_tensor(f"y_out_{i}", [DI, TOK], F16, kind="Internal"))
    Tn["xia_in"], Tn["xia_out"] = xia_in, xia_out
    Tn["xd_in"], Tn["xd_out"] = xd_in, xd_out
    Tn["y_in"], Tn["y_out"] = y_in, y_out

    with tile.TileContext(nc) as tc:
        _emit(nc, tc, Tn, has_lnb, has_nfb)

    nc.compile()
    return nc


def _emit(nc, tc, Tn, has_lnb, has_nfb):
    import contextlib
    RG = [list(range(NCORES))]

    def load_table(set_id):
        nc.scalar.add_instruction(mybir.InstLoadActFuncSet(
            name=nc.get_next_instruction_name(), ins=[], outs=[],
            act_func_set_id=set_id))

    ctx = contextlib.ExitStack()
    with ctx:
        consts = ctx.enter_context(tc.tile_pool(name="consts", bufs=1))
        wpool = ctx.enter_context(tc.tile_pool(name="wpool", bufs=1))
        small = ctx.enter_context(tc.tile_pool(name="small", bufs=2))
        xnpool = ctx.enter_context(tc.tile_pool(name="xnpool", bufs=1))
        ygpool = ctx.enter_context(tc.tile_pool(name="ygpool", bufs=1))
        stats = ctx.enter_context(tc.tile_pool(name="stats", bufs=1))
        bigs = ctx.enter_context(tc.tile_pool(name="bigs", bufs=1))
        zpool = ctx.enter_context(tc.tile_pool(name="zpool", bufs=1))
        xcur_p = ctx.enter_context(tc.tile_pool(name="xcur", bufs=1))
        spool = ctx.enter_context(tc.tile_pool(name="spool", bufs=2))
        hpool = ctx.enter_context(tc.tile_pool(name="hpool", bufs=1))
        ytpool = ctx.enter_context(tc.tile_pool(name="ytpool", bufs=1))
        opool = ctx.enter_context(tc.tile_pool(name="opool", bufs=1))
        rowpool = ctx.enter_context(tc.tile_pool(name="rowpool", bufs=2))
        bcast = ctx.enter_context(tc.tile_pool(name="bcast", bufs=3))
        evac = ctx.enter_context(tc.tile_pool(name="evac", bufs=2))
        ps_mm = ctx.enter_context(tc.tile_pool(name="ps_mm", bufs=2, space="PSUM"))
        ps_bc = ctx.enter_context(tc.tile_pool(name="ps_bc", bufs=2, space="PSUM"))
        ps_y = ctx.enter_context(tc.tile_pool(name="ps_y", bufs=1, space="PSUM"))

        def mm(out, lhsT, rhs, **kw):
            nc.tensor.matmul(out, lhsT=lhsT, rhs=rhs, **kw)

        # ------------- constants -------------
        ident = consts.tile([128, 128], F16, tag="ident")
        nc.sync.dma_start(out=ident[:], in_=Tn["identin"].ap())
        onescol = consts.tile([128, 1], F32R, tag="onescol")
        nc.vector.memset(onescol[:].bitcast(F32), 1.0)
        ones1 = consts.tile([1, 128], F32R, tag="ones1")
        nc.vector.memset(ones1[:].bitcast(F32), 1.0)
        ones16 = consts.tile([1, CH], F16, tag="ones16")
        nc.vector.memset(ones16[:], 1.0)
        ones1h = consts.tile([1, 128], F16, tag="ones1h")
        nc.vector.memset(ones1h[:], 1.0)
        eps_sb = consts.tile([128, 1], F32, tag="eps")
        nc.vector.memset(eps_sb[:], EPS)
        nfw_sb = consts.tile([128, NG], F32, tag="nfw")
        nc.sync.dma_start(out=nfw_sb[:], in_=Tn["nfw"].ap())
        nfb_sb = consts.tile([128, NG], F32, tag="nfb")
        nc.sync.dma_start(out=nfb_sb[:], in_=Tn["nfb"].ap())

        load_table(LN_EXP_SET)

        def load_x0():
            xt = []
            for g in range(NG):
                xg = xcur_p.tile([128, TOK], F32R, tag=f"xcur{g}",
                                 name=f"xcur{g}_init")
                nc.sync.dma_start(
                    out=xg[:],
                    in_=Tn["xs"].ap()[128 * g:128 * (g + 1), :].bitcast(F32R))
                xt.append(xg)
            return xt

        x_cur = load_x0()

        def ln_head(x_tiles):
            """stats -> (m_row f32r, rstd_row f32r, mrs16 f16)"""
            s1 = ps_bc.tile([1, TOK], F32, tag="bc", name="s1")
            s2 = ps_bc.tile([1, TOK], F32, tag="bc", name="s2")
            for g in range(NG):
                xsq = small.tile([128, TOK], F32R, tag="xsq")
                nc.scalar.square(out=xsq[:], in_=x_tiles[g][:].bitcast(F32))
                mm(s1[:], lhsT=onescol[:], rhs=x_tiles[g][:],
                   start=(g == 0), stop=(g == NG - 1))
                mm(s2[:], lhsT=onescol[:], rhs=xsq[:],
                   start=(g == 0), stop=(g == NG - 1))
            m_row = stats.tile([1, TOK], F32R, tag="mrow")
            nc.vector.tensor_scalar_mul(out=m_row[:], in0=s1[:],
                                        scalar1=1.0 / D)
            mu2 = small.tile([1, TOK], F32, tag="mu2")
            nc.vector.tensor_mul(out=mu2[:], in0=m_row[:].bitcast(F32),
                                 in1=m_row[:].bitcast(F32))
            var_row = stats.tile([1, TOK], F32, tag="var")
            nc.vector.scalar_tensor_tensor(
                out=var_row[:], in0=s2[:], scalar=1.0 / D, in1=mu2[:],
                op0=Alu.mult, op1=Alu.subtract)
            nc.scalar.activation(out=var_row[:], in_=var_row[:],
                                 func=Act.Ln, bias=eps_sb[:1, :])
            rstd_row = stats.tile([1, TOK], F32R, tag="rstd")
            nc.scalar.activation(out=rstd_row[:], in_=var_row[:],
                                 func=Act.Exp, scale=-0.5)
            return m_row, rstd_row

        # ---------------- per-block loop ----------------
        for i in range(NB):
            rev = (i % 2 == 1)
            wi_sb = wpool.tile([128, 64 * 128], F16, tag="wi")
            nc.sync.dma_start(out=wi_sb[:], in_=Tn["wi"].ap()[i])
            negrs_sb = wpool.tile([1, 16 * 128], F16, tag="negrs")
            nc.sync.dma_start(out=negrs_sb[:], in_=Tn["negrs"].ap()[i])
            lnb_sb = None
            if has_lnb:
                lnb_sb = wpool.tile([1, 16 * 128], F16, tag="lnb")
                nc.sync.dma_start(out=lnb_sb[:], in_=Tn["lnbias"].ap()[i])
            convd_sb = wpool.tile([128, KCONV * 128], F16, tag="convd")
            nc.sync.dma_start(out=convd_sb[:], in_=Tn["convd"].ap()[i])
            convb_sb = wpool.tile([128, 1], F32, tag="convb")
            nc.sync.dma_start(out=convb_sb[:], in_=Tn["convb"].ap()[i])
            wx_sb = wpool.tile([128, 64], F16, tag="wx")
            nc.sync.dma_start(out=wx_sb[:], in_=Tn["wx"].ap()[i])
            wdt_sb = wpool.tile([32, 128], F16, tag="wdt")
            nc.sync.dma_start(out=wdt_sb[:], in_=Tn["wdt"].ap()[i])
            bdt_sb = wpool.tile([1, 128], F16, tag="bdt")
            nc.sync.dma_start(out=bdt_sb[:], in_=Tn["bdt"].ap()[i])
            acols_sb = wpool.tile([128, NST], F32, tag="acols")
            nc.sync.dma_start(out=acols_sb[:], in_=Tn["acols"].ap()[i])
            dpcol_sb = wpool.tile([128, 1], F32, tag="dpcol")
            nc.sync.dma_start(out=dpcol_sb[:], in_=Tn["dpcol"].ap()[i])
            wo_sb = wpool.tile([128, 32 * 128], F16, tag="wo")
            nc.sync.dma_start(out=wo_sb[:], in_=Tn["wo"].ap()[i])

            # ---- T1: LN + in-proj (token-shard), xi groups then z ----
            m_row, rstd_row = ln_head(x_cur)
            mrs16 = stats.tile([1, TOK], F16, tag="mrs16")
            nc.vector.tensor_mul(out=mrs16[:], in0=m_row[:].bitcast(F32),
                                 in1=rstd_row[:].bitcast(F32))
            rbc = ps_bc.tile([128, TOK], F32, tag="bc", name="rbc")
            mm(rbc[:], lhsT=ones1[:], rhs=rstd_row[:], start=True, stop=True)
            xn = []
            for g in range(NG):
                xng = xnpool.tile([128, TOK], F16, tag=f"xn{g}",
                                  name=f"xn{g}")
                nc.vector.tensor_mul(out=xng[:],
                                     in0=x_cur[g][:].bitcast(F32),
                                     in1=rbc[:])
                xn.append(xng)

            def inproj_group(g, psname):
                xz = ps_mm.tile([128, TOK], F32, tag="mm", name=psname)
                for kc in range(NG):
                    lh = wi_sb[:, (g * 4 + kc) * 128:(g * 4 + kc + 1) * 128]
                    mm(xz[:], lhsT=lh, rhs=xn[kc][:],
                       start=(kc == 0), stop=False)
                mm(xz[:], lhsT=negrs_sb[:, g * 128:(g + 1) * 128],
                   rhs=mrs16[:], start=False, stop=(not has_lnb))
                if has_lnb:
                    mm(xz[:], lhsT=lnb_sb[:, g * 128:(g + 1) * 128],
                       rhs=ones16[:], start=False, stop=True)
                return xz

            for g in range(NDI):          # xi rows
                xz = inproj_group(g, f"xz_xi{g}")
                xi16 = evac.tile([128, TOK], F16, tag="xi16")
                nc.scalar.copy(out=xi16[:], in_=xz[:])
                nc.sync.dma_start(
                    out=Tn["xia_in"][i].ap()[128 * g:128 * (g + 1), :],
                    in_=xi16[:])
            nc.gpsimd.collective_compute(
                "AllToAll", Alu.bypass, replica_groups=RG,
                ins=[Tn["xia_in"][i].ap()], outs=[Tn["xia_out"][i].ap()])
            load_table(SILU_SET)
            zs = []
            for g in range(NDI):          # z rows -> silu, kept in SBUF
                xz = inproj_group(NDI + g, f"xz_z{g}")
                zg = zpool.tile([128, TOK], F16, tag=f"zs{g}", name=f"zs{g}")
                nc.scalar.activation(out=zg[:], in_=xz[:], func=Act.Silu)
                zs.append(zg)

            # ---- C1: conv + Wx + AR + dt + scan (channel-shard) ----
            xipad = [bigs.tile([128, L + 6], F16, tag=f"xipad{b}",
                               name=f"xipad{b}") for b in range(B)]
            xis = [bigs.tile([128, L], F16, tag=f"xis{b}", name=f"xis{b}")
                   for b in range(B)]
            for b in range(B):
                nc.vector.memset(xipad[b][:, 0:3], 0.0)
                nc.vector.memset(xipad[b][:, L + 3:L + 6], 0.0)
            for s in range(NCORES):
                b, q = s // NCH, s % NCH
                nc.sync.dma_start(
                    out=xipad[b][:, 3 + CH * q: 3 + CH * (q + 1)],
                    in_=Tn["xia_out"][i].ap()[128 * s:128 * (s + 1), :])

            for b in range(B):
                for c in range(NCH):
                    t0 = c * CH
                    cv = ps_mm.tile([128, CH], F32, tag="mm")
                    for kk in range(KCONV):
                        off = t0 + (kk if not rev else (6 - kk))
                        mm(cv[:], lhsT=convd_sb[:, kk * 128:(kk + 1) * 128],
                           rhs=xipad[b][:, off: off + CH],
                           start=(kk == 0), stop=(kk == KCONV - 1))
                    nc.scalar.activation(out=xis[b][:, t0:t0 + CH], in_=cv[:],
                                         func=Act.Silu, bias=convb_sb[:])
                    wxp = ps_mm.tile([64, CH], F32, tag="mm", name="wxp")
                    mm(wxp[:], lhsT=wx_sb[:], rhs=xis[b][:, t0:t0 + CH],
                       start=True, stop=True)
                    xd16c = evac.tile([64, CH], F16, tag="xd16c")
                    nc.scalar.copy(out=xd16c[:], in_=wxp[:])
                    nc.sync.dma_start(
                        out=Tn["xd_in"][i].ap()[:, b * L + t0:b * L + t0 + CH],
                        in_=xd16c[:])
            xd_rs, xd_ag = Tn["xd_out"][i]
            nc.gpsimd.collective_compute(
                "ReduceScatter", Alu.add, replica_groups=RG,
                ins=[Tn["xd_in"][i].ap()], outs=[xd_rs.ap()])
            nc.gpsimd.collective_compute(
                "AllGather", Alu.bypass, replica_groups=RG,
                ins=[xd_rs.ap()], outs=[xd_ag.ap()])

            dtr32 = bigs.tile([32, T], F16, tag="dtr32", name="dtr32")
            for sl in range(NCORES):
                nc.sync.dma_start(
                    out=dtr32[:, TOK * sl:TOK * (sl + 1)],
                    in_=xd_ag.ap()[sl, 0:32, :])

            load_table(LN_EXP_SET)
            dt = [bigs.tile([128, L], F16, tag=f"dt{b}", name=f"dt{b}")
                  for b in range(B)]
            dtx = [bigs.tile([128, L], F16, tag=f"dtx{b}", name=f"dtx{b}")
                   for b in range(B)]
            for b in range(B):
                for c in range(NCH):
                    t0 = c * CH
                    gt0 = b * L + t0
                    dt_ps = ps_mm.tile([128, CH], F32, tag="mm",
                                       name="dt_ps")
                    mm(dt_ps[:], lhsT=wdt_sb[:], rhs=dtr32[:, gt0:gt0 + CH],
                       start=True, stop=False)
                    mm(dt_ps[:], lhsT=bdt_sb[:], rhs=ones16[:],
                       start=False, stop=True)
                    e_sb = small.tile([128, CH], F16, tag="sp_e", name="e_sb")
                    nc.scalar.activation(out=e_sb[:], in_=dt_ps[:],
                                         func=Act.Exp, scale=-1.0)
                    nc.scalar.activation(out=e_sb[:], in_=e_sb[:],
                                         func=Act.Ln, bias=1.0)
                    dtr = small.tile([128, CH], F16, tag="dtr", name="dtr")
                    nc.scalar.copy(out=dtr[:], in_=dt_ps[:])
                    nc.vector.tensor_add(out=dt[b][:, t0:t0 + CH],
                                         in0=dtr[:], in1=e_sb[:])
                    nc.vector.tensor_mul(out=dtx[b][:, t0:t0 + CH],
                                         in0=dt[b][:, t0:t0 + CH],
                                         in1=xis[b][:, t0:t0 + CH])

            # ---- scan: full-batch per state; Pool scans, DVE muls ----
            for b in range(B):
                y_ps = [ps_y.tile([128, CH], F32, tag=f"y{c}", name=f"y{c}")
                        for c in range(NCH)]

                def revfull(tl):
                    return tl[:, L - 1::-1] if rev else tl[:, 0:L]

                def revchunk(tl, t0o):
                    if not rev:
                        return tl[:, t0o:t0o + CH]
                    if t0o == 0:
                        return tl[:, CH - 1::-1]
                    return tl[:, t0o + CH - 1:t0o - 1:-1]

                for n in range(NST):
                    act_route = (n % 2 == 1)  # C-side via PE rank-1 + Act evac
                    dA = spool.tile([128, L], F16, tag="dA")
                    nc.scalar.activation(out=dA[:], in_=revfull(dt[b]),
                                         func=Act.Exp,
                                         scale=acols_sb[:, n:n + 1])
                    brow = rowpool.tile([1, L], F16, tag="brow")
                    nc.sync.dma_start(
                        out=brow[:],
                        in_=Tn["xd_out"][i][1].ap()[
                            NCH * b:NCH * (b + 1), 32 + n:33 + n, :])
                    dBu = spool.tile([128, L], F16, tag="dBu")
                    bbc_sb = bcast.tile([128, L], F16, tag="bbc")
                    nc.gpsimd.partition_broadcast(bbc_sb[:], brow[0:1, :])
                    nc.vector.tensor_mul(out=dBu[:], in0=revfull(dtx[b]),
                                         in1=revfull(bbc_sb))
                    h = hpool.tile([128, L], F16, tag="h")
                    nc.vector.tensor_tensor_scan(h[:], dA[:], dBu[:], 0.0,
                                                 op0=Alu.mult, op1=Alu.add)
                    crow = rowpool.tile([1, L], F16, tag="crow")
                    nc.sync.dma_start(
                        out=crow[:],
                        in_=Tn["xd_out"][i][1].ap()[
                            NCH * b:NCH * (b + 1), 48 + n:49 + n, :])
                    cbc_sb = bcast.tile([128, L], F16, tag="cbc")
                    if act_route:
                        for c in range(NCH):
                            cbc = ps_bc.tile([128, CH], F32, tag="bc")
                            mm(cbc[:], lhsT=ones1h[:],
                               rhs=crow[0:1, c * CH:(c + 1) * CH],
                               start=True, stop=True)
                            nc.scalar.copy(
                                out=cbc_sb[:, c * CH:(c + 1) * CH],
                                in_=cbc[:])
                    else:
                        nc.gpsimd.partition_broadcast(cbc_sb[:], crow[0:1, :])
                    yterm = ytpool.tile([128, L], F16, tag="yterm")
                    nc.vector.tensor_mul(out=yterm[:], in0=h[:],
                                         in1=revfull(cbc_sb))
                    for c in range(NCH):
                        mm(y_ps[c][:], lhsT=ident[:],
                           rhs=yterm[:, c * CH:(c + 1) * CH],
                           start=(n == 0), stop=(n == NST - 1))
                for c in range(NCH):
                    co = (NCH - 1 - c) if rev else c
                    t0o = co * CH
                    gc = NCH * b + co
                    y16 = evac.tile([128, CH], F16, tag="y16")
                    yout = y16[:, CH - 1::-1] if rev else y16[:]
                    nc.scalar.copy(out=yout, in_=y_ps[c][:])
                    yfin = evac.tile([128, CH], F16, tag="yfin")
                    nc.vector.scalar_tensor_tensor(
                        out=yfin[:], in0=xis[b][:, t0o:t0o + CH],
                        scalar=dpcol_sb[:], in1=y16[:],
                        op0=Alu.mult, op1=Alu.add)
                    nc.sync.dma_start(
                        out=Tn["y_in"][i].ap()[128 * gc:128 * (gc + 1), :],
                        in_=yfin[:])
            nc.gpsimd.collective_compute(
                "AllToAll", Alu.bypass, replica_groups=RG,
                ins=[Tn["y_in"][i].ap()], outs=[Tn["y_out"][i].ap()])

            # ---- T2: gate + out-proj (token-shard) ----
            yg = []
            for s in range(NDI):
                ydn = evac.tile([128, TOK], F16, tag="ydn")
                nc.sync.dma_start(
                    out=ydn[:],
                    in_=Tn["y_out"][i].ap()[128 * s:128 * (s + 1), :])
                ygs = ygpool.tile([128, TOK], F16, tag=f"yg{s}",
                                  name=f"yg{s}")
                nc.vector.tensor_mul(out=ygs[:], in0=ydn[:], in1=zs[s][:])
                yg.append(ygs)
            x_next = []
            for g in range(NG):
                op_ps = ps_mm.tile([128, TOK], F32, tag="mm")
                for kc in range(NDI):
                    lh = wo_sb[:, (g * 8 + kc) * 128:(g * 8 + kc + 1) * 128]
                    mm(op_ps[:], lhsT=lh, rhs=yg[kc][:],
                       start=(kc == 0), stop=(kc == NDI - 1))
                xg = xcur_p.tile([128, TOK], F32R, tag=f"xcur{g}",
                                 name=f"xcur{g}_{i}")
                nc.scalar.copy(out=xg[:], in_=op_ps[:])
                x_next.append(xg)
            x_cur = x_next

        # ---------------- final layernorm (token-shard) ----------------
        m_row, rstd_row = ln_head(x_cur)
        mbc = ps_bc.tile([128, TOK], F32, tag="bc", name="mbc")
        mm(mbc[:], lhsT=ones1[:], rhs=m_row[:], start=True, stop=True)
        rbc_ps = ps_bc.tile([128, TOK], F32, tag="bc", name="rbcf")
        mm(rbc_ps[:], lhsT=ones1[:], rhs=rstd_row[:], start=True, stop=True)
        rbc = small.tile([128, TOK], F32, tag="rbc")
        nc.scalar.copy(out=rbc[:], in_=rbc_ps[:])
        for g in range(NG):
            t1_sb = small.tile([128, TOK], F32, tag="xsq", name="t1_sb")
            nc.vector.tensor_sub(out=t1_sb[:],
                                 in0=x_cur[g][:].bitcast(F32), in1=mbc[:])
            o_sb = opool.tile([128, TOK], F32, tag="o_sb")
            nc.vector.scalar_tensor_tensor(
                out=o_sb[:], in0=t1_sb[:], scalar=nfw_sb[:, g:g + 1],
                in1=rbc[:], op0=Alu.mult, op1=Alu.mult)
            if has_nfb:
                nc.vector.tensor_scalar_add(
                    out=o_sb[:], in0=o_sb[:], scalar1=nfb_sb[:, g:g + 1])
            nc.sync.dma_start(
                out=Tn["outs"].ap()[128 * g:128 * (g + 1), :], in_=o_sb[:])


def _host_prep(inputs):
    x = np.asarray(inputs["x"], np.float32)
    ln_w = np.asarray(inputs["ln_w"], np.float32)
    ln_b = np.asarray(inputs["ln_b"], np.float32)
    W_in = np.asarray(inputs["W_in"], np.float32)
    conv_w = np.asarray(inputs["conv_w"], np.float32)
    conv_b = np.asarray(inputs["conv_b"], np.float32)
    W_x = np.asarray(inputs["W_x"], np.float32)
    W_dt = np.asarray(inputs["W_dt"], np.float32)
    b_dt = np.asarray(inputs["b_dt"], np.float32)
    A_log = np.asarray(inputs["A_log"], np.float32)
    D_p = np.asarray(inputs["D_p"], np.float32)
    W_out = np.asarray(inputs["W_out"], np.float32)
    normf_w = np.asarray(inputs["normf_w"], np.float32)
    normf_b = np.asarray(inputs["normf_b"], np.float32)

    xT = np.ascontiguousarray(x.transpose(2, 0, 1).reshape(D, T))
    A = -np.exp(A_log)

    wi_arr = np.zeros((NB, 128, 64 * 128), np.float16)
    negrs_arr = np.zeros((NB, 1, 16 * 128), np.float16)
    lnb_arr = np.zeros((NB, 1, 16 * 128), np.float16)
    wo_arr = np.zeros((NB, 128, 32 * 128), np.float16)
    for i in range(NB):
        Wf = W_in[i] * ln_w[i][None, :]
        for g in range(16):
            rows = slice(128 * g, 128 * (g + 1))
            for kc in range(4):
                cols = slice(128 * kc, 128 * (kc + 1))
                wi_arr[i, :, (g * 4 + kc) * 128:(g * 4 + kc + 1) * 128] = \
                    Wf[rows, cols].T
            negrs_arr[i, 0, g * 128:(g + 1) * 128] = -Wf[rows, :].sum(1)
            lnb_arr[i, 0, g * 128:(g + 1) * 128] = W_in[i][rows, :] @ ln_b[i]
        for g in range(4):
            rows = slice(128 * g, 128 * (g + 1))
            for kc in range(8):
                cols = slice(128 * kc, 128 * (kc + 1))
                wo_arr[i, :, (g * 8 + kc) * 128:(g * 8 + kc + 1) * 128] = \
                    W_out[i][rows, cols].T

    selbc = np.zeros((64, 32 * 128), np.float16)
    for q in range(32):
        selbc[32 + q, q * 128:(q + 1) * 128] = 1.0

    nfw = np.ascontiguousarray(normf_w.reshape(NG, 128).T)
    nfb = np.ascontiguousarray(normf_b.reshape(NG, 128).T)
    identin = np.eye(128, dtype=np.float16)

    in_maps = []
    for k in range(NCORES):
        sl = slice(128 * k, 128 * (k + 1))
        convd_arr = np.zeros((NB, 128, KCONV * 128), np.float16)
        convb_arr = np.zeros((NB, 128, 1), np.float32)
        wx_arr = np.zeros((NB, 128, 64), np.float16)
        wdt_arr = np.zeros((NB, 32, 128), np.float16)
        bdt_arr = np.zeros((NB, 1, 128), np.float16)
        acols_arr = np.zeros((NB, 128, NST), np.float32)
        dp_arr = np.zeros((NB, 128, 1), np.float32)
        for i in range(NB):
            for kk in range(KCONV):
                np.fill_diagonal(convd_arr[i, :, kk * 128:(kk + 1) * 128],
                                 conv_w[i, sl, kk])
            convb_arr[i, :, 0] = conv_b[i, sl]
            wx_arr[i] = W_x[i][:, sl].T
            wdt_arr[i] = W_dt[i][sl, :].T
            bdt_arr[i, 0, :] = b_dt[i, sl]
            acols_arr[i] = A[i, sl, :]
            dp_arr[i, :, 0] = D_p[i, sl]
        in_maps.append({
            "xs": np.ascontiguousarray(xT[:, TOK * k:TOK * (k + 1)]),
            "wi": wi_arr, "negrs": negrs_arr, "lnbias": lnb_arr,
            "convd": convd_arr, "convb": convb_arr,
            "wx": wx_arr, "wdt": wdt_arr, "bdt": bdt_arr,
            "acols": acols_arr, "dpcol": dp_arr, "wo": wo_arr,
            "nfw": nfw, "nfb": nfb, "identin": identin, "selbc": selbc,
        })
    has_lnb = bool(np.any(ln_b != 0.0))
    has_nfb = bool(np.any(normf_b != 0.0))
    return in_maps, has_lnb, has_nfb


def _get_program(has_lnb, has_nfb):
    key = (has_lnb, has_nfb)
    if key not in _PROGRAM_CACHE:
        _PROGRAM_CACHE[key] = _build_program(has_lnb, has_nfb)
    return _PROGRAM_CACHE[key]


def kernel(**inputs) -> np.ndarray:
    global _LAST_RESULTS
    in_maps, has_lnb, has_nfb = _host_prep(inputs)
    nc = _get_program(has_lnb, has_nfb)
    res = bass_utils.run_bass_kernel_spmd(nc, in_maps,
                                          core_ids=list(range(NCORES)))
    _LAST_RESULTS = res
    out_T = np.concatenate([res.results[k]["outs"] for k in range(NCORES)],
                           axis=1)
    out = out_T.reshape(D, B, L).transpose(1, 2, 0)
    return np.ascontiguousarray(out.astype(np.float32))
